# revision 9
# baseline (speedup 1.0000x reference)
"""Trainium2 Bass kernel for nn_MultiHeadAttention_69088843923801.

Key structural fact: the reference reshapes (B, T, nh*H) -> (B, nh, T, H) as a
raw row-major reinterpretation.  Head n therefore only ever touches x rows
[n*256, (n+1)*256), and the whole layer decomposes into B*nh = 32 fully
independent 256-row blocks (attention is the only cross-row op and it stays
inside a block; LN/FFN are row-wise).  We run 4 blocks per NeuronCore on 8
cores: pure data parallelism, no collectives, no redundant compute.

Per block (x_n = 256 rows of x):
  A_q = x_n @ Wq  -> Qg = A_q.reshape(2048, 256)   (same for K, V)
  S   = Qg @ Kg.T / 16 ; A = softmax(S) ; Og = A @ Vg
  h_attn = Og.reshape(256, 2048) @ Wo
  h = LN(x_n + h_attn); out_rows = LN(h + relu(h@W1 + bb1)@W2 + bb2)

On-chip we enumerate the 2048 "derived tokens" as k' = c*256 + r (c = column
block of the 2048-wide projection, r = row in the block) which makes every
matmul operand a contiguous slice.  Matmuls run in float32r (TF32-like, 11-bit
mantissa, full PE rate at N>=256); accumulation is fp32 in PSUM.

The emission order software-pipelines blocks: block j+1's Q/K projections are
emitted inside block j's LayerNorm1 window so the PE never idles on the LN
serial chain.
"""

import sys

sys.path.insert(0, "/opt/trn_rl_repo")

import numpy as np

N_CORES = 8
B, T, H, NH = 4, 2048, 256, 8
BLOCKS_PER_CORE = 4
EPS = 1e-5
SCALE = 0.0625  # 1/sqrt(H)

_CACHE = {}
LAST_RESULTS = None
TRACE_TMPDIR = None


def _round_f32r(x):
    """Round fp32 to float32r (11 explicit mantissa bits, RNE) like the DVE does."""
    u = np.ascontiguousarray(x, dtype=np.float32).view(np.uint32)
    low = u & np.uint32(0xFFF)
    up = (low > 0x800) | ((low == 0x800) & (((u >> np.uint32(12)) & np.uint32(1)) == 1))
    u = (u & np.uint32(0xFFFFF000)) + np.where(up, np.uint32(0x1000), np.uint32(0)).astype(np.uint32)
    return u.view(np.float32)


def _build():
    import concourse.bacc as bacc
    import concourse.tile as tile
    import concourse.mybir as mybir

    AF = mybir.ActivationFunctionType
    OP = mybir.AluOpType
    AX = mybir.AxisListType
    F32 = mybir.dt.float32
    F32R = mybir.dt.float32r

    nc = bacc.Bacc("TRN2", target_bir_lowering=False, debug=False, num_devices=N_CORES)

    d = {}
    d["xt"] = nc.dram_tensor("xt", [256, 1024], F32R, kind="ExternalInput").ap()
    d["wq"] = nc.dram_tensor("wq", [256, 2048], F32R, kind="ExternalInput").ap()
    d["wk"] = nc.dram_tensor("wk", [256, 2048], F32R, kind="ExternalInput").ap()
    d["wv"] = nc.dram_tensor("wv", [256, 2048], F32R, kind="ExternalInput").ap()
    d["wo"] = nc.dram_tensor("wo", [2048, 256], F32R, kind="ExternalInput").ap()
    d["w1"] = nc.dram_tensor("w1", [256, 1024], F32R, kind="ExternalInput").ap()
    d["w2"] = nc.dram_tensor("w2", [1024, 256], F32R, kind="ExternalInput").ap()
    d["onesm"] = nc.dram_tensor("onesm", [128, 128], F32R, kind="ExternalInput").ap()
    d["ident"] = nc.dram_tensor("ident", [128, 128], F32, kind="ExternalInput").ap()
    d["g1c"] = nc.dram_tensor("g1c", [128, 2], F32, kind="ExternalInput").ap()
    d["b1c"] = nc.dram_tensor("b1c", [128, 2], F32, kind="ExternalInput").ap()
    d["bb1c"] = nc.dram_tensor("bb1c", [128, 8], F32, kind="ExternalInput").ap()
    d["bb2c"] = nc.dram_tensor("bb2c", [128, 2], F32, kind="ExternalInput").ap()
    d["epsc"] = nc.dram_tensor("epsc", [128, 1], F32, kind="ExternalInput").ap()
    d["g2r"] = nc.dram_tensor("g2r", [1, 256], F32R, kind="ExternalInput").ap()
    d["b2r"] = nc.dram_tensor("b2r", [1, 256], F32R, kind="ExternalInput").ap()
    out_d = nc.dram_tensor("out", [1024, 256], F32, kind="ExternalOutput").ap()

    with tile.TileContext(nc) as tc:
        with tc.tile_pool(name="wts", bufs=1) as wp, \
             tc.tile_pool(name="blk", bufs=1) as bp, \
             tc.tile_pool(name="ep", bufs=3) as ep, \
             tc.tile_pool(name="rlp", bufs=3) as rlp, \
             tc.tile_pool(name="rbp", bufs=2) as rbp, \
             tc.tile_pool(name="stp", bufs=1) as stp, \
             tc.tile_pool(name="psA", bufs=2, space="PSUM") as psA, \
             tc.tile_pool(name="psS", bufs=3, space="PSUM") as psS, \
             tc.tile_pool(name="psO", bufs=1, space="PSUM") as psO:

            # ---------------- loads: small consts first so PE starts fast ----
            ones = wp.tile([128, 128], F32R, tag="ones", name="ones")
            nc.sync.dma_start(ones[:], d["onesm"][:])
            ident = wp.tile([128, 128], F32, tag="ident", name="ident")
            nc.sync.dma_start(ident[:], d["ident"][:])
            g1c = wp.tile([128, 2], F32, tag="g1c", name="g1c")
            b1c = wp.tile([128, 2], F32, tag="b1c", name="b1c")
            bb1c = wp.tile([128, 8], F32, tag="bb1c", name="bb1c")
            bb2c = wp.tile([128, 2], F32, tag="bb2c", name="bb2c")
            nc.sync.dma_start(g1c[:], d["g1c"][:])
            nc.sync.dma_start(b1c[:], d["b1c"][:])
            nc.sync.dma_start(bb1c[:], d["bb1c"][:])
            nc.sync.dma_start(bb2c[:], d["bb2c"][:])
            epst = wp.tile([128, 1], F32, tag="epst", name="epst")
            nc.sync.dma_start(epst[:], d["epsc"][:])
            g2r = wp.tile([1, 256], F32R, tag="g2r", name="g2r")
            b2r = wp.tile([1, 256], F32R, tag="b2r", name="b2r")
            nc.sync.dma_start(g2r[:], d["g2r"][:])
            nc.sync.dma_start(b2r[:], d["b2r"][:])

            xt = [wp.tile([128, 1024], F32R, tag=f"xt{i}", name=f"xt{i}") for i in range(2)]
            wq = [wp.tile([128, 2048], F32R, tag=f"wq{i}", name=f"wq{i}") for i in range(2)]
            wk = [wp.tile([128, 2048], F32R, tag=f"wk{i}", name=f"wk{i}") for i in range(2)]
            wv = [wp.tile([128, 2048], F32R, tag=f"wv{i}", name=f"wv{i}") for i in range(2)]
            for i in range(2):
                nc.sync.dma_start(xt[i][:, 0:256], d["xt"][i * 128:(i + 1) * 128, 0:256])
            for c in range(8):
                for i in range(2):
                    nc.sync.dma_start(wq[i][:, c * 256:(c + 1) * 256],
                                      d["wq"][i * 128:(i + 1) * 128, c * 256:(c + 1) * 256])
                    nc.sync.dma_start(wk[i][:, c * 256:(c + 1) * 256],
                                      d["wk"][i * 128:(i + 1) * 128, c * 256:(c + 1) * 256])
            for i in range(2):
                nc.sync.dma_start(xt[i][:, 256:1024], d["xt"][i * 128:(i + 1) * 128, 256:1024])
            for i in range(2):
                nc.sync.dma_start(wv[i][:], d["wv"][i * 128:(i + 1) * 128, :])

            # broadcast g2/b2 across partitions once: ones[0:1,:].T @ row
            g2b = wp.tile([128, 256], F32, tag="g2b", name="g2b")
            b2b = wp.tile([128, 256], F32, tag="b2b", name="b2b")
            for row, dst in ((g2r, g2b), (b2r, b2b)):
                pb = psA.tile([128, 256], F32, tag="mm", name="mm")
                nc.tensor.matmul(pb[:], ones[0:1, :], row[:], start=True, stop=True)
                nc.vector.tensor_copy(dst[:], pb[:])

            # bulkier weights, needed only from the Wo / FFN phase onwards
            wo = wp.tile([128, 4096], F32R, tag="wo", name="wo")
            for kc in range(16):
                nc.sync.dma_start(wo[:, kc * 256:(kc + 1) * 256],
                                  d["wo"][kc * 128:(kc + 1) * 128, :])
            w1 = [wp.tile([128, 1024], F32R, tag=f"w1{i}", name=f"w1{i}") for i in range(2)]
            for i in range(2):
                nc.sync.dma_start(w1[i][:], d["w1"][i * 128:(i + 1) * 128, :])
            w2 = wp.tile([128, 2048], F32R, tag="w2", name="w2")
            for ncc in range(8):
                nc.sync.dma_start(w2[:, ncc * 256:(ncc + 1) * 256],
                                  d["w2"][ncc * 128:(ncc + 1) * 128, :])

            blocks = {}

            def qkv_qtkt(j):
                xcol = j * 256
                qt = [bp.tile([128, 2048], F32R, tag=f"qt{i}", name=f"qt{i}") for i in range(2)]
                kt = [bp.tile([128, 2048], F32R, tag=f"kt{i}", name=f"kt{i}") for i in range(2)]
                blocks[j] = {"qt": qt, "kt": kt}
                for c in range(8):
                    for hc in range(2):
                        col = c * 256 + hc * 128
                        pq = psA.tile([128, 256], F32, tag="mm", name="mm")
                        for hic in range(2):
                            nc.tensor.matmul(pq[:], wq[hic][:, col:col + 128],
                                             xt[hic][:, xcol:xcol + 256],
                                             start=(hic == 0), stop=(hic == 1))
                        nc.vector.tensor_copy(qt[hc][:, c * 256:(c + 1) * 256], pq[:])
                        pk = psA.tile([128, 256], F32, tag="mm", name="mm")
                        for hic in range(2):
                            nc.tensor.matmul(pk[:], wk[hic][:, col:col + 128],
                                             xt[hic][:, xcol:xcol + 256],
                                             start=(hic == 0), stop=(hic == 1))
                        nc.vector.tensor_copy(kt[hc][:, c * 256:(c + 1) * 256], pk[:])

            def qkv_av(j):
                xcol = j * 256
                av = [bp.tile([128, 2048], F32R, tag=f"av{i}", name=f"av{i}") for i in range(2)]
                blocks[j]["av"] = av
                for rt in range(2):
                    for nc4 in range(4):
                        pv = psA.tile([128, 512], F32, tag="mm", name="mm")
                        for hic in range(2):
                            nc.tensor.matmul(pv[:], xt[hic][:, xcol + rt * 128:xcol + rt * 128 + 128],
                                             wv[hic][:, nc4 * 512:(nc4 + 1) * 512],
                                             start=(hic == 0), stop=(hic == 1))
                        nc.vector.tensor_copy(av[rt][:, nc4 * 512:(nc4 + 1) * 512], pv[:])

            def attention(j):
                qt, kt, av = blocks[j]["qt"], blocks[j]["kt"], blocks[j]["av"]
                onorm = [bp.tile([128, 2048], F32R, tag=f"on{i}", name=f"on{i}") for i in range(2)]
                blocks[j]["onorm"] = onorm
                for qc in range(4):
                    o0 = psO.tile([128, 512], F32, tag="o0", name="o0")
                    o1 = psO.tile([128, 512], F32, tag="o1", name="o1")
                    dn = psO.tile([128, 512], F32, tag="d", name="d")
                    es = {}

                    def avd(k):
                        c_, rt_ = k // 2, k % 2
                        e = es.pop(k)
                        nc.tensor.matmul(dn[:], ones[:], e[:],
                                         start=(k == 0), stop=(k == 15))
                        nc.tensor.matmul(o0[:], av[rt_][:, c_ * 256:c_ * 256 + 128], e[:],
                                         start=(k == 0), stop=(k == 15))
                        nc.tensor.matmul(o1[:], av[rt_][:, c_ * 256 + 128:(c_ + 1) * 256], e[:],
                                         start=(k == 0), stop=(k == 15))

                    # software-pipelined: S/exp run 2 k-tiles ahead of the
                    # dependent AV/denominator matmuls so PE never waits on ACT
                    for kc in range(16):
                        s = psS.tile([128, 512], F32, tag="s", name="s")
                        nc.tensor.matmul(s[:], kt[0][:, kc * 128:(kc + 1) * 128],
                                         qt[0][:, qc * 512:(qc + 1) * 512],
                                         start=True, stop=False)
                        nc.tensor.matmul(s[:], kt[1][:, kc * 128:(kc + 1) * 128],
                                         qt[1][:, qc * 512:(qc + 1) * 512],
                                         start=False, stop=True)
                        e = ep.tile([128, 512], F32R, tag="e", name="e")
                        nc.scalar.activation(e[:], s[:], AF.Exp, scale=SCALE)
                        es[kc] = e
                        if kc >= 2:
                            avd(kc - 2)
                    avd(14)
                    avd(15)
                    rb = rbp.tile([128, 512], F32, tag="rb", name="rb")
                    nc.vector.reciprocal(rb[:], dn[:])
                    nc.vector.tensor_tensor(onorm[0][:, qc * 512:(qc + 1) * 512], o0[:], rb[:], op=OP.mult)
                    nc.vector.tensor_tensor(onorm[1][:, qc * 512:(qc + 1) * 512], o1[:], rb[:], op=OP.mult)

            def wo_z1(j):
                xcol = j * 256
                onorm = blocks[j]["onorm"]
                z1 = [bp.tile([128, 256], F32R, tag=f"z1{i}", name=f"z1{i}") for i in range(2)]
                blocks[j]["z1"] = z1
                for hoc in range(2):
                    ph = psA.tile([128, 256], F32, tag="mm", name="mm")
                    for kc in range(16):
                        c_, h2c = kc // 2, kc % 2
                        nc.tensor.matmul(ph[:], wo[:, kc * 256 + hoc * 128:kc * 256 + hoc * 128 + 128],
                                         onorm[h2c][:, c_ * 256:(c_ + 1) * 256],
                                         start=(kc == 0), stop=(kc == 15))
                    nc.vector.tensor_add(z1[hoc][:], ph[:], xt[hoc][:, xcol:xcol + 256])

            def ln1_stats(j):
                z1 = blocks[j]["z1"]
                sq = [bp.tile([128, 256], F32R, tag=f"sq{i}", name=f"sq{i}") for i in range(2)]
                nc.scalar.square(sq[0][:], z1[0][:])
                nc.scalar.square(sq[1][:], z1[1][:])
                ssum = psA.tile([128, 256], F32, tag="mm", name="mm")
                nc.tensor.matmul(ssum[:], ones[:], z1[0][:], start=True, stop=False)
                nc.tensor.matmul(ssum[:], ones[:], z1[1][:], start=False, stop=True)
                ssq = psA.tile([128, 256], F32, tag="mm", name="mm")
                nc.tensor.matmul(ssq[:], ones[:], sq[0][:], start=True, stop=False)
                nc.tensor.matmul(ssq[:], ones[:], sq[1][:], start=False, stop=True)
                blocks[j]["ssum"] = ssum
                blocks[j]["ssq"] = ssq

            def ln1_norm(j):
                z1 = blocks[j]["z1"]
                ssum, ssq = blocks[j]["ssum"], blocks[j]["ssq"]
                mu_b = stp.tile([128, 256], F32, tag="mu", name="mu")
                nc.vector.tensor_scalar_mul(mu_b[:], ssum[:], 1.0 / 256.0)
                mu2 = stp.tile([128, 256], F32, tag="mu2", name="mu2")
                nc.vector.tensor_tensor(mu2[:], mu_b[:], mu_b[:], op=OP.mult)
                var = stp.tile([128, 256], F32, tag="var", name="var")
                nc.vector.scalar_tensor_tensor(var[:], ssq[:], 1.0 / 256.0, mu2[:],
                                               op0=OP.mult, op1=OP.subtract)
                sd = stp.tile([128, 256], F32, tag="sd", name="sd")
                nc.scalar.activation(sd[:], var[:], AF.Sqrt, bias=epst[:])
                rsg = stp.tile([128, 256], F32, tag="rsg", name="rsg")
                nc.vector.reciprocal(rsg[:], sd[:])
                hn = [bp.tile([128, 256], F32R, tag=f"hn{i}", name=f"hn{i}") for i in range(2)]
                blocks[j]["hn"] = hn
                for hc in range(2):
                    t1 = stp.tile([128, 256], F32, tag="t1", name="t1")
                    nc.vector.tensor_tensor(t1[:], z1[hc][:], mu_b[:], op=OP.subtract)
                    t2 = stp.tile([128, 256], F32, tag="t2", name="t2")
                    nc.vector.tensor_tensor(t2[:], t1[:], rsg[:], op=OP.mult)
                    nc.vector.tensor_scalar(hn[hc][:], t2[:],
                                            g1c[:, hc:hc + 1], b1c[:, hc:hc + 1],
                                            op0=OP.mult, op1=OP.add)

            def ffn(j):
                hn = blocks[j]["hn"]
                z2p = [psO.tile([128, 256], F32, tag=t, name=t) for t in ("o0", "o1")]
                for ncc in range(8):
                    pf = psS.tile([128, 256], F32, tag="s", name="s")
                    for hic in range(2):
                        nc.tensor.matmul(pf[:], w1[hic][:, ncc * 128:(ncc + 1) * 128],
                                         hn[hic][:], start=(hic == 0), stop=(hic == 1))
                    rl = rlp.tile([128, 256], F32R, tag="rl", name="rl")
                    nc.scalar.activation(rl[:], pf[:], AF.Relu, bias=bb1c[:, ncc:ncc + 1])
                    nc.tensor.matmul(z2p[0][:], w2[:, ncc * 256:ncc * 256 + 128], rl[:],
                                     start=(ncc == 0), stop=(ncc == 7))
                    nc.tensor.matmul(z2p[1][:], w2[:, ncc * 256 + 128:(ncc + 1) * 256], rl[:],
                                     start=(ncc == 0), stop=(ncc == 7))
                z2 = [bp.tile([128, 256], F32, tag=f"z2{i}", name=f"z2{i}") for i in range(2)]
                blocks[j]["z2"] = z2
                for hoc in range(2):
                    nc.vector.scalar_tensor_tensor(z2[hoc][:], z2p[hoc][:],
                                                   bb2c[:, hoc:hoc + 1], hn[hoc][:],
                                                   op0=OP.add, op1=OP.add)

            def ln2_out(j):
                z2 = blocks[j]["z2"]
                for rt in range(2):
                    zr = stp.tile([128, 256], F32, tag="zr", name="zr")
                    for hoc in range(2):
                        pt = psS.tile([128, 128], F32, tag="s", name="s")
                        nc.tensor.transpose(pt[:], z2[hoc][:, rt * 128:(rt + 1) * 128], ident[:])
                        nc.vector.tensor_copy(zr[:, hoc * 128:(hoc + 1) * 128], pt[:])
                    srow = stp.tile([128, 1], F32, tag="srow", name="srow")
                    nc.vector.reduce_sum(srow[:], zr[:], axis=AX.X)
                    sqs = stp.tile([128, 256], F32, tag="sqs", name="sqs")
                    ssqr = stp.tile([128, 1], F32, tag="ssqr", name="ssqr")
                    nc.scalar.activation(sqs[:], zr[:], AF.Square, accum_out=ssqr[:])
                    mur = stp.tile([128, 1], F32, tag="mur", name="mur")
                    nc.vector.tensor_scalar_mul(mur[:], srow[:], 1.0 / 256.0)
                    mu2r = stp.tile([128, 1], F32, tag="mu2r", name="mu2r")
                    nc.vector.tensor_tensor(mu2r[:], mur[:], mur[:], op=OP.mult)
                    varr = stp.tile([128, 1], F32, tag="varr", name="varr")
                    nc.vector.scalar_tensor_tensor(varr[:], ssqr[:], 1.0 / 256.0, mu2r[:],
                                                   op0=OP.mult, op1=OP.subtract)
                    sdr = stp.tile([128, 1], F32, tag="sdr", name="sdr")
                    nc.scalar.activation(sdr[:], varr[:], AF.Sqrt, bias=epst[:])
                    rsr = stp.tile([128, 1], F32, tag="rsr", name="rsr")
                    nc.vector.reciprocal(rsr[:], sdr[:])
                    tt = stp.tile([128, 256], F32, tag="tt", name="tt")
                    nc.vector.tensor_scalar(tt[:], zr[:], mur[:], rsr[:],
                                            op0=OP.subtract, op1=OP.mult)
                    tg = stp.tile([128, 256], F32, tag="tg", name="tg")
                    nc.vector.tensor_tensor(tg[:], tt[:], g2b[:], op=OP.mult)
                    ot = stp.tile([128, 256], F32, tag="ot", name="ot")
                    nc.vector.tensor_tensor(ot[:], tg[:], b2b[:], op=OP.add)
                    nc.sync.dma_start(out_d[j * 256 + rt * 128:j * 256 + (rt + 1) * 128, :], ot[:])

            # ---------------- schedule ----------------
            # Emission order = per-engine program order.  LN1's serial chain is
            # emitted BEFORE block j+1's QKV copies so it sits early on the DVE
            # queue and finishes while the PE chews on the QKV matmuls; the
            # next attention is emitted before ln2_out so the transposes never
            # head-block the PE.
            qkv_qtkt(0)
            qkv_av(0)
            attention(0)
            for j in range(BLOCKS_PER_CORE):
                wo_z1(j)
                ln1_stats(j)
                ln1_norm(j)
                if j + 1 < BLOCKS_PER_CORE:
                    qkv_qtkt(j + 1)
                    qkv_av(j + 1)
                ffn(j)
                if j + 1 < BLOCKS_PER_CORE:
                    attention(j + 1)
                ln2_out(j)

    nc.compile()
    return nc


def kernel(x, Wq, Wk, Wv, Wo, g1, b1, W1, bb1, W2, bb2, g2, b2):
    from concourse import bass_utils
    global LAST_RESULTS

    if "nc" not in _CACHE:
        _CACHE["nc"] = _build()
    nc = _CACHE["nc"]

    x = np.ascontiguousarray(np.asarray(x, dtype=np.float32))
    shared = {
        "wq": _round_f32r(np.asarray(Wq, np.float32)),
        "wk": _round_f32r(np.asarray(Wk, np.float32)),
        "wv": _round_f32r(np.asarray(Wv, np.float32)),
        "wo": _round_f32r(np.asarray(Wo, np.float32)),
        "w1": _round_f32r(np.asarray(W1, np.float32)),
        "w2": _round_f32r(np.asarray(W2, np.float32)),
        "onesm": np.ones((128, 128), np.float32),
        "ident": np.eye(128, dtype=np.float32),
        "g1c": np.ascontiguousarray(np.asarray(g1, np.float32).reshape(2, 128).T),
        "b1c": np.ascontiguousarray(np.asarray(b1, np.float32).reshape(2, 128).T),
        "bb1c": np.ascontiguousarray(np.asarray(bb1, np.float32).reshape(8, 128).T),
        "bb2c": np.ascontiguousarray(np.asarray(bb2, np.float32).reshape(2, 128).T),
        "epsc": np.full((128, 1), EPS, np.float32),
        "g2r": _round_f32r(np.asarray(g2, np.float32).reshape(1, 256)),
        "b2r": _round_f32r(np.asarray(b2, np.float32).reshape(1, 256)),
    }

    in_maps = []
    for c in range(N_CORES):
        xt = np.empty((256, 1024), np.float32)
        for j in range(BLOCKS_PER_CORE):
            g = c * BLOCKS_PER_CORE + j
            b_, n_ = g // NH, g % NH
            xt[:, j * 256:(j + 1) * 256] = x[b_, n_ * 256:(n_ + 1) * 256, :].T
        m = dict(shared)
        m["xt"] = _round_f32r(xt)
        in_maps.append(m)

    kwargs = {}
    if TRACE_TMPDIR is not None:
        kwargs["tmpdir"] = TRACE_TMPDIR
    res = bass_utils.run_bass_kernel_spmd(nc, in_maps, core_ids=list(range(N_CORES)), **kwargs)
    LAST_RESULTS = res

    out = np.empty((B, T, H), np.float32)
    for c in range(N_CORES):
        o = res.results[c]["out"]
        for j in range(BLOCKS_PER_CORE):
            g = c * BLOCKS_PER_CORE + j
            b_, n_ = g // NH, g % NH
            out[b_, n_ * 256:(n_ + 1) * 256, :] = o[j * 256:(j + 1) * 256, :]
    return out


# revision 10
# speedup vs baseline: 1.0222x; 1.0222x over previous
"""Trainium2 Bass kernel for nn_MultiHeadAttention_69088843923801.

Key structural fact: the reference reshapes (B, T, nh*H) -> (B, nh, T, H) as a
raw row-major reinterpretation.  Head n therefore only ever touches x rows
[n*256, (n+1)*256), and the whole layer decomposes into B*nh = 32 fully
independent 256-row blocks (attention is the only cross-row op and it stays
inside a block; LN/FFN are row-wise).  We run 4 blocks per NeuronCore on 8
cores: pure data parallelism, no collectives, no redundant compute.

Per block (x_n = 256 rows of x):
  A_q = x_n @ Wq  -> Qg = A_q.reshape(2048, 256)   (same for K, V)
  S   = Qg @ Kg.T / 16 ; A = softmax(S) ; Og = A @ Vg
  h_attn = Og.reshape(256, 2048) @ Wo
  h = LN(x_n + h_attn); out_rows = LN(h + relu(h@W1 + bb1)@W2 + bb2)

On-chip we enumerate the 2048 "derived tokens" as k' = c*256 + r (c = column
block of the 2048-wide projection, r = row in the block) which makes every
matmul operand a contiguous slice.  Matmuls run in float32r (TF32-like, 11-bit
mantissa, full PE rate at N>=256); accumulation is fp32 in PSUM.

The emission order software-pipelines blocks: block j+1's Q/K projections are
emitted inside block j's LayerNorm1 window so the PE never idles on the LN
serial chain.
"""

import sys

sys.path.insert(0, "/opt/trn_rl_repo")

import numpy as np

N_CORES = 8
B, T, H, NH = 4, 2048, 256, 8
BLOCKS_PER_CORE = 4
EPS = 1e-5
SCALE = 0.0625  # 1/sqrt(H)

_CACHE = {}
LAST_RESULTS = None
TRACE_TMPDIR = None


def _round_f32r(x):
    """Round fp32 to float32r (11 explicit mantissa bits, RNE) like the DVE does."""
    u = np.ascontiguousarray(x, dtype=np.float32).view(np.uint32)
    low = u & np.uint32(0xFFF)
    up = (low > 0x800) | ((low == 0x800) & (((u >> np.uint32(12)) & np.uint32(1)) == 1))
    u = (u & np.uint32(0xFFFFF000)) + np.where(up, np.uint32(0x1000), np.uint32(0)).astype(np.uint32)
    return u.view(np.float32)


def _build():
    import concourse.bacc as bacc
    import concourse.tile as tile
    import concourse.mybir as mybir

    AF = mybir.ActivationFunctionType
    OP = mybir.AluOpType
    AX = mybir.AxisListType
    F32 = mybir.dt.float32
    F32R = mybir.dt.float32r

    nc = bacc.Bacc("TRN2", target_bir_lowering=False, debug=False, num_devices=N_CORES)

    d = {}
    d["xt"] = nc.dram_tensor("xt", [256, 1024], F32R, kind="ExternalInput").ap()
    d["wq"] = nc.dram_tensor("wq", [256, 2048], F32R, kind="ExternalInput").ap()
    d["wk"] = nc.dram_tensor("wk", [256, 2048], F32R, kind="ExternalInput").ap()
    d["wv"] = nc.dram_tensor("wv", [256, 2048], F32R, kind="ExternalInput").ap()
    d["wo"] = nc.dram_tensor("wo", [2048, 256], F32R, kind="ExternalInput").ap()
    d["w1"] = nc.dram_tensor("w1", [256, 1024], F32R, kind="ExternalInput").ap()
    d["w2"] = nc.dram_tensor("w2", [1024, 256], F32R, kind="ExternalInput").ap()
    d["onesm"] = nc.dram_tensor("onesm", [128, 128], F32R, kind="ExternalInput").ap()
    d["ident"] = nc.dram_tensor("ident", [128, 128], F32, kind="ExternalInput").ap()
    d["g1c"] = nc.dram_tensor("g1c", [128, 2], F32, kind="ExternalInput").ap()
    d["b1c"] = nc.dram_tensor("b1c", [128, 2], F32, kind="ExternalInput").ap()
    d["bb1c"] = nc.dram_tensor("bb1c", [128, 8], F32, kind="ExternalInput").ap()
    d["bb2c"] = nc.dram_tensor("bb2c", [128, 2], F32, kind="ExternalInput").ap()
    d["epsc"] = nc.dram_tensor("epsc", [128, 1], F32, kind="ExternalInput").ap()
    d["g2r"] = nc.dram_tensor("g2r", [1, 256], F32R, kind="ExternalInput").ap()
    d["b2r"] = nc.dram_tensor("b2r", [1, 256], F32R, kind="ExternalInput").ap()
    out_d = nc.dram_tensor("out", [1024, 256], F32, kind="ExternalOutput").ap()

    with tile.TileContext(nc) as tc:
        with tc.tile_pool(name="wts", bufs=1) as wp, \
             tc.tile_pool(name="blk", bufs=1) as bp, \
             tc.tile_pool(name="ep", bufs=4) as ep, \
             tc.tile_pool(name="rlp", bufs=3) as rlp, \
             tc.tile_pool(name="rbp", bufs=2) as rbp, \
             tc.tile_pool(name="stp", bufs=1) as stp, \
             tc.tile_pool(name="psA", bufs=2, space="PSUM") as psA, \
             tc.tile_pool(name="psS", bufs=3, space="PSUM") as psS, \
             tc.tile_pool(name="psO", bufs=1, space="PSUM") as psO:

            # ---------------- loads: small consts first so PE starts fast ----
            ones = wp.tile([128, 128], F32R, tag="ones", name="ones")
            nc.sync.dma_start(ones[:], d["onesm"][:])
            ident = wp.tile([128, 128], F32, tag="ident", name="ident")
            nc.sync.dma_start(ident[:], d["ident"][:])
            g1c = wp.tile([128, 2], F32, tag="g1c", name="g1c")
            b1c = wp.tile([128, 2], F32, tag="b1c", name="b1c")
            bb1c = wp.tile([128, 8], F32, tag="bb1c", name="bb1c")
            bb2c = wp.tile([128, 2], F32, tag="bb2c", name="bb2c")
            nc.sync.dma_start(g1c[:], d["g1c"][:])
            nc.sync.dma_start(b1c[:], d["b1c"][:])
            nc.sync.dma_start(bb1c[:], d["bb1c"][:])
            nc.sync.dma_start(bb2c[:], d["bb2c"][:])
            epst = wp.tile([128, 1], F32, tag="epst", name="epst")
            nc.sync.dma_start(epst[:], d["epsc"][:])
            g2r = wp.tile([1, 256], F32R, tag="g2r", name="g2r")
            b2r = wp.tile([1, 256], F32R, tag="b2r", name="b2r")
            nc.sync.dma_start(g2r[:], d["g2r"][:])
            nc.sync.dma_start(b2r[:], d["b2r"][:])

            xt = [wp.tile([128, 1024], F32R, tag=f"xt{i}", name=f"xt{i}") for i in range(2)]
            wq = [wp.tile([128, 2048], F32R, tag=f"wq{i}", name=f"wq{i}") for i in range(2)]
            wk = [wp.tile([128, 2048], F32R, tag=f"wk{i}", name=f"wk{i}") for i in range(2)]
            wv = [wp.tile([128, 2048], F32R, tag=f"wv{i}", name=f"wv{i}") for i in range(2)]
            for i in range(2):
                nc.sync.dma_start(xt[i][:, 0:256], d["xt"][i * 128:(i + 1) * 128, 0:256])
            for c in range(8):
                for i in range(2):
                    nc.sync.dma_start(wq[i][:, c * 256:(c + 1) * 256],
                                      d["wq"][i * 128:(i + 1) * 128, c * 256:(c + 1) * 256])
                    nc.sync.dma_start(wk[i][:, c * 256:(c + 1) * 256],
                                      d["wk"][i * 128:(i + 1) * 128, c * 256:(c + 1) * 256])
            for i in range(2):
                nc.sync.dma_start(xt[i][:, 256:1024], d["xt"][i * 128:(i + 1) * 128, 256:1024])
            for i in range(2):
                nc.sync.dma_start(wv[i][:], d["wv"][i * 128:(i + 1) * 128, :])

            # broadcast g2/b2 across partitions once: ones[0:1,:].T @ row
            g2b = wp.tile([128, 256], F32, tag="g2b", name="g2b")
            b2b = wp.tile([128, 256], F32, tag="b2b", name="b2b")
            for row, dst in ((g2r, g2b), (b2r, b2b)):
                pb = psA.tile([128, 256], F32, tag="mm", name="mm")
                nc.tensor.matmul(pb[:], ones[0:1, :], row[:], start=True, stop=True)
                nc.vector.tensor_copy(dst[:], pb[:])

            # bulkier weights, needed only from the Wo / FFN phase onwards
            wo = wp.tile([128, 4096], F32R, tag="wo", name="wo")
            for kc in range(16):
                nc.sync.dma_start(wo[:, kc * 256:(kc + 1) * 256],
                                  d["wo"][kc * 128:(kc + 1) * 128, :])
            w1 = [wp.tile([128, 1024], F32R, tag=f"w1{i}", name=f"w1{i}") for i in range(2)]
            for i in range(2):
                nc.sync.dma_start(w1[i][:], d["w1"][i * 128:(i + 1) * 128, :])
            w2 = wp.tile([128, 2048], F32R, tag="w2", name="w2")
            for ncc in range(8):
                nc.sync.dma_start(w2[:, ncc * 256:(ncc + 1) * 256],
                                  d["w2"][ncc * 128:(ncc + 1) * 128, :])

            blocks = {}

            def qkv_qtkt(j):
                xcol = j * 256
                # per-chunk tiles -> fine-grained dependencies (consumers only
                # wait for the specific chunk's producer, not the whole tensor)
                qt = [[bp.tile([128, 512], F32R, tag=f"qt{i}_{qc}", name=f"qt{i}_{qc}")
                       for qc in range(4)] for i in range(2)]
                kt = [[bp.tile([128, 256], F32R, tag=f"kt{i}_{c}", name=f"kt{i}_{c}")
                       for c in range(8)] for i in range(2)]
                blocks[j] = {"qt": qt, "kt": kt}
                for c in range(8):
                    for hc in range(2):
                        col = c * 256 + hc * 128
                        pq = psA.tile([128, 256], F32, tag="mm", name="mm")
                        for hic in range(2):
                            nc.tensor.matmul(pq[:], wq[hic][:, col:col + 128],
                                             xt[hic][:, xcol:xcol + 256],
                                             start=(hic == 0), stop=(hic == 1))
                        nc.vector.tensor_copy(qt[hc][c // 2][:, (c % 2) * 256:(c % 2) * 256 + 256], pq[:])
                        pk = psA.tile([128, 256], F32, tag="mm", name="mm")
                        for hic in range(2):
                            nc.tensor.matmul(pk[:], wk[hic][:, col:col + 128],
                                             xt[hic][:, xcol:xcol + 256],
                                             start=(hic == 0), stop=(hic == 1))
                        nc.vector.tensor_copy(kt[hc][c][:], pk[:])

            def qkv_av(j):
                xcol = j * 256
                av = [[bp.tile([128, 512], F32R, tag=f"av{i}_{nc4}", name=f"av{i}_{nc4}")
                       for nc4 in range(4)] for i in range(2)]
                blocks[j]["av"] = av
                for rt in range(2):
                    for nc4 in range(4):
                        pv = psA.tile([128, 512], F32, tag="mm", name="mm")
                        for hic in range(2):
                            nc.tensor.matmul(pv[:], xt[hic][:, xcol + rt * 128:xcol + rt * 128 + 128],
                                             wv[hic][:, nc4 * 512:(nc4 + 1) * 512],
                                             start=(hic == 0), stop=(hic == 1))
                        nc.vector.tensor_copy(av[rt][nc4][:], pv[:])

            def attention(j):
                qt, kt, av = blocks[j]["qt"], blocks[j]["kt"], blocks[j]["av"]
                onorm = [[bp.tile([128, 512], F32R, tag=f"on{i}_{qc}", name=f"on{i}_{qc}")
                          for qc in range(4)] for i in range(2)]
                blocks[j]["onorm"] = onorm
                for qc in range(4):
                    o0 = psO.tile([128, 512], F32, tag="o0", name="o0")
                    o1 = psO.tile([128, 512], F32, tag="o1", name="o1")
                    dn = psO.tile([128, 512], F32, tag="d", name="d")
                    es = {}

                    def avd(k):
                        c_, rt_ = k // 2, k % 2
                        e = es.pop(k)
                        acol = c_ * 256 + 0 - (c_ // 2) * 512
                        nc.tensor.matmul(dn[:], ones[:], e[:],
                                         start=(k == 0), stop=(k == 15))
                        nc.tensor.matmul(o0[:], av[rt_][c_ // 2][:, acol:acol + 128], e[:],
                                         start=(k == 0), stop=(k == 15))
                        nc.tensor.matmul(o1[:], av[rt_][c_ // 2][:, acol + 128:acol + 256], e[:],
                                         start=(k == 0), stop=(k == 15))

                    # software-pipelined: S/exp run 3 k-tiles ahead of the
                    # dependent AV/denominator matmuls so PE never waits on ACT
                    for kc in range(16):
                        s = psS.tile([128, 512], F32, tag="s", name="s")
                        nc.tensor.matmul(s[:], kt[0][kc // 2][:, (kc % 2) * 128:(kc % 2) * 128 + 128],
                                         qt[0][qc][:],
                                         start=True, stop=False)
                        nc.tensor.matmul(s[:], kt[1][kc // 2][:, (kc % 2) * 128:(kc % 2) * 128 + 128],
                                         qt[1][qc][:],
                                         start=False, stop=True)
                        e = ep.tile([128, 512], F32R, tag="e", name="e")
                        nc.scalar.activation(e[:], s[:], AF.Exp, scale=SCALE)
                        es[kc] = e
                        if kc >= 3:
                            avd(kc - 3)
                    avd(13)
                    avd(14)
                    avd(15)
                    rb = rbp.tile([128, 512], F32, tag="rb", name="rb")
                    nc.vector.reciprocal(rb[:], dn[:])
                    nc.vector.tensor_tensor(onorm[0][qc][:], o0[:], rb[:], op=OP.mult)
                    nc.vector.tensor_tensor(onorm[1][qc][:], o1[:], rb[:], op=OP.mult)

            def wo_z1(j):
                xcol = j * 256
                onorm = blocks[j]["onorm"]
                z1 = [bp.tile([128, 256], F32R, tag=f"z1{i}", name=f"z1{i}") for i in range(2)]
                blocks[j]["z1"] = z1
                for hoc in range(2):
                    ph = psA.tile([128, 256], F32, tag="mm", name="mm")
                    for kc in range(16):
                        c_, h2c = kc // 2, kc % 2
                        ocol = (c_ % 2) * 256
                        nc.tensor.matmul(ph[:], wo[:, kc * 256 + hoc * 128:kc * 256 + hoc * 128 + 128],
                                         onorm[h2c][c_ // 2][:, ocol:ocol + 256],
                                         start=(kc == 0), stop=(kc == 15))
                    nc.vector.tensor_add(z1[hoc][:], ph[:], xt[hoc][:, xcol:xcol + 256])

            def ln1_stats(j):
                z1 = blocks[j]["z1"]
                sq = [bp.tile([128, 256], F32R, tag=f"sq{i}", name=f"sq{i}") for i in range(2)]
                nc.scalar.square(sq[0][:], z1[0][:])
                nc.scalar.square(sq[1][:], z1[1][:])
                ssum = psA.tile([128, 256], F32, tag="mm", name="mm")
                nc.tensor.matmul(ssum[:], ones[:], z1[0][:], start=True, stop=False)
                nc.tensor.matmul(ssum[:], ones[:], z1[1][:], start=False, stop=True)
                ssq = psA.tile([128, 256], F32, tag="mm", name="mm")
                nc.tensor.matmul(ssq[:], ones[:], sq[0][:], start=True, stop=False)
                nc.tensor.matmul(ssq[:], ones[:], sq[1][:], start=False, stop=True)
                blocks[j]["ssum"] = ssum
                blocks[j]["ssq"] = ssq

            def ln1_norm(j):
                z1 = blocks[j]["z1"]
                ssum, ssq = blocks[j]["ssum"], blocks[j]["ssq"]
                mu_b = stp.tile([128, 256], F32, tag="mu", name="mu")
                nc.vector.tensor_scalar_mul(mu_b[:], ssum[:], 1.0 / 256.0)
                mu2 = stp.tile([128, 256], F32, tag="mu2", name="mu2")
                nc.vector.tensor_tensor(mu2[:], mu_b[:], mu_b[:], op=OP.mult)
                var = stp.tile([128, 256], F32, tag="var", name="var")
                nc.vector.scalar_tensor_tensor(var[:], ssq[:], 1.0 / 256.0, mu2[:],
                                               op0=OP.mult, op1=OP.subtract)
                sd = stp.tile([128, 256], F32, tag="sd", name="sd")
                nc.scalar.activation(sd[:], var[:], AF.Sqrt, bias=epst[:])
                rsg = stp.tile([128, 256], F32, tag="rsg", name="rsg")
                nc.vector.reciprocal(rsg[:], sd[:])
                hn = [bp.tile([128, 256], F32R, tag=f"hn{i}", name=f"hn{i}") for i in range(2)]
                blocks[j]["hn"] = hn
                for hc in range(2):
                    t1 = stp.tile([128, 256], F32, tag="t1", name="t1")
                    nc.vector.tensor_tensor(t1[:], z1[hc][:], mu_b[:], op=OP.subtract)
                    t2 = stp.tile([128, 256], F32, tag="t2", name="t2")
                    nc.vector.tensor_tensor(t2[:], t1[:], rsg[:], op=OP.mult)
                    nc.vector.tensor_scalar(hn[hc][:], t2[:],
                                            g1c[:, hc:hc + 1], b1c[:, hc:hc + 1],
                                            op0=OP.mult, op1=OP.add)

            def ffn(j):
                hn = blocks[j]["hn"]
                z2p = [psO.tile([128, 256], F32, tag=t, name=t) for t in ("o0", "o1")]
                for ncc in range(8):
                    pf = psS.tile([128, 256], F32, tag="s", name="s")
                    for hic in range(2):
                        nc.tensor.matmul(pf[:], w1[hic][:, ncc * 128:(ncc + 1) * 128],
                                         hn[hic][:], start=(hic == 0), stop=(hic == 1))
                    rl = rlp.tile([128, 256], F32R, tag="rl", name="rl")
                    nc.scalar.activation(rl[:], pf[:], AF.Relu, bias=bb1c[:, ncc:ncc + 1])
                    nc.tensor.matmul(z2p[0][:], w2[:, ncc * 256:ncc * 256 + 128], rl[:],
                                     start=(ncc == 0), stop=(ncc == 7))
                    nc.tensor.matmul(z2p[1][:], w2[:, ncc * 256 + 128:(ncc + 1) * 256], rl[:],
                                     start=(ncc == 0), stop=(ncc == 7))
                z2 = [bp.tile([128, 256], F32, tag=f"z2{i}", name=f"z2{i}") for i in range(2)]
                blocks[j]["z2"] = z2
                for hoc in range(2):
                    nc.vector.scalar_tensor_tensor(z2[hoc][:], z2p[hoc][:],
                                                   bb2c[:, hoc:hoc + 1], hn[hoc][:],
                                                   op0=OP.add, op1=OP.add)

            def ln2_out(j):
                z2 = blocks[j]["z2"]
                for rt in range(2):
                    zr = stp.tile([128, 256], F32, tag="zr", name="zr")
                    for hoc in range(2):
                        pt = psS.tile([128, 128], F32, tag="s", name="s")
                        nc.tensor.transpose(pt[:], z2[hoc][:, rt * 128:(rt + 1) * 128], ident[:])
                        nc.vector.tensor_copy(zr[:, hoc * 128:(hoc + 1) * 128], pt[:])
                    srow = stp.tile([128, 1], F32, tag="srow", name="srow")
                    nc.vector.reduce_sum(srow[:], zr[:], axis=AX.X)
                    sqs = stp.tile([128, 256], F32, tag="sqs", name="sqs")
                    ssqr = stp.tile([128, 1], F32, tag="ssqr", name="ssqr")
                    nc.scalar.activation(sqs[:], zr[:], AF.Square, accum_out=ssqr[:])
                    mur = stp.tile([128, 1], F32, tag="mur", name="mur")
                    nc.vector.tensor_scalar_mul(mur[:], srow[:], 1.0 / 256.0)
                    mu2r = stp.tile([128, 1], F32, tag="mu2r", name="mu2r")
                    nc.vector.tensor_tensor(mu2r[:], mur[:], mur[:], op=OP.mult)
                    varr = stp.tile([128, 1], F32, tag="varr", name="varr")
                    nc.vector.scalar_tensor_tensor(varr[:], ssqr[:], 1.0 / 256.0, mu2r[:],
                                                   op0=OP.mult, op1=OP.subtract)
                    sdr = stp.tile([128, 1], F32, tag="sdr", name="sdr")
                    nc.scalar.activation(sdr[:], varr[:], AF.Sqrt, bias=epst[:])
                    rsr = stp.tile([128, 1], F32, tag="rsr", name="rsr")
                    nc.vector.reciprocal(rsr[:], sdr[:])
                    tt = stp.tile([128, 256], F32, tag="tt", name="tt")
                    nc.vector.tensor_scalar(tt[:], zr[:], mur[:], rsr[:],
                                            op0=OP.subtract, op1=OP.mult)
                    tg = stp.tile([128, 256], F32, tag="tg", name="tg")
                    nc.vector.tensor_tensor(tg[:], tt[:], g2b[:], op=OP.mult)
                    ot = stp.tile([128, 256], F32, tag="ot", name="ot")
                    nc.vector.tensor_tensor(ot[:], tg[:], b2b[:], op=OP.add)
                    nc.sync.dma_start(out_d[j * 256 + rt * 128:j * 256 + (rt + 1) * 128, :], ot[:])

            # ---------------- schedule ----------------
            # Emission order = per-engine program order.  LN1's serial chain is
            # emitted BEFORE block j+1's QKV copies so it sits early on the DVE
            # queue and finishes while the PE chews on the QKV matmuls; the
            # next attention is emitted before ln2_out so the transposes never
            # head-block the PE.
            qkv_qtkt(0)
            qkv_av(0)
            attention(0)
            for j in range(BLOCKS_PER_CORE):
                wo_z1(j)
                ln1_stats(j)
                ln1_norm(j)
                if j + 1 < BLOCKS_PER_CORE:
                    qkv_qtkt(j + 1)
                    qkv_av(j + 1)
                ffn(j)
                if j + 1 < BLOCKS_PER_CORE:
                    attention(j + 1)
                ln2_out(j)

    nc.compile()
    return nc


def kernel(x, Wq, Wk, Wv, Wo, g1, b1, W1, bb1, W2, bb2, g2, b2):
    from concourse import bass_utils
    global LAST_RESULTS

    if "nc" not in _CACHE:
        _CACHE["nc"] = _build()
    nc = _CACHE["nc"]

    x = np.ascontiguousarray(np.asarray(x, dtype=np.float32))
    shared = {
        "wq": _round_f32r(np.asarray(Wq, np.float32)),
        "wk": _round_f32r(np.asarray(Wk, np.float32)),
        "wv": _round_f32r(np.asarray(Wv, np.float32)),
        "wo": _round_f32r(np.asarray(Wo, np.float32)),
        "w1": _round_f32r(np.asarray(W1, np.float32)),
        "w2": _round_f32r(np.asarray(W2, np.float32)),
        "onesm": np.ones((128, 128), np.float32),
        "ident": np.eye(128, dtype=np.float32),
        "g1c": np.ascontiguousarray(np.asarray(g1, np.float32).reshape(2, 128).T),
        "b1c": np.ascontiguousarray(np.asarray(b1, np.float32).reshape(2, 128).T),
        "bb1c": np.ascontiguousarray(np.asarray(bb1, np.float32).reshape(8, 128).T),
        "bb2c": np.ascontiguousarray(np.asarray(bb2, np.float32).reshape(2, 128).T),
        "epsc": np.full((128, 1), EPS, np.float32),
        "g2r": _round_f32r(np.asarray(g2, np.float32).reshape(1, 256)),
        "b2r": _round_f32r(np.asarray(b2, np.float32).reshape(1, 256)),
    }

    in_maps = []
    for c in range(N_CORES):
        xt = np.empty((256, 1024), np.float32)
        for j in range(BLOCKS_PER_CORE):
            g = c * BLOCKS_PER_CORE + j
            b_, n_ = g // NH, g % NH
            xt[:, j * 256:(j + 1) * 256] = x[b_, n_ * 256:(n_ + 1) * 256, :].T
        m = dict(shared)
        m["xt"] = _round_f32r(xt)
        in_maps.append(m)

    kwargs = {}
    if TRACE_TMPDIR is not None:
        kwargs["tmpdir"] = TRACE_TMPDIR
    res = bass_utils.run_bass_kernel_spmd(nc, in_maps, core_ids=list(range(N_CORES)), **kwargs)
    LAST_RESULTS = res

    out = np.empty((B, T, H), np.float32)
    for c in range(N_CORES):
        o = res.results[c]["out"]
        for j in range(BLOCKS_PER_CORE):
            g = c * BLOCKS_PER_CORE + j
            b_, n_ = g // NH, g % NH
            out[b_, n_ * 256:(n_ + 1) * 256, :] = o[j * 256:(j + 1) * 256, :]
    return out


# revision 11
# speedup vs baseline: 1.0356x; 1.0131x over previous
"""Trainium2 Bass kernel for nn_MultiHeadAttention_69088843923801.

Key structural fact: the reference reshapes (B, T, nh*H) -> (B, nh, T, H) as a
raw row-major reinterpretation.  Head n therefore only ever touches x rows
[n*256, (n+1)*256), and the whole layer decomposes into B*nh = 32 fully
independent 256-row blocks (attention is the only cross-row op and it stays
inside a block; LN/FFN are row-wise).  We run 4 blocks per NeuronCore on 8
cores: pure data parallelism, no collectives, no redundant compute.

Per block (x_n = 256 rows of x):
  A_q = x_n @ Wq  -> Qg = A_q.reshape(2048, 256)   (same for K, V)
  S   = Qg @ Kg.T / 16 ; A = softmax(S) ; Og = A @ Vg
  h_attn = Og.reshape(256, 2048) @ Wo
  h = LN(x_n + h_attn); out_rows = LN(h + relu(h@W1 + bb1)@W2 + bb2)

On-chip we enumerate the 2048 "derived tokens" as k' = c*256 + r (c = column
block of the 2048-wide projection, r = row in the block) which makes every
matmul operand a contiguous slice.  Matmuls run in float32r (TF32-like, 11-bit
mantissa, full PE rate at N>=256); accumulation is fp32 in PSUM.

The emission order software-pipelines blocks: block j+1's Q/K projections are
emitted inside block j's LayerNorm1 window so the PE never idles on the LN
serial chain.
"""

import sys

sys.path.insert(0, "/opt/trn_rl_repo")

import numpy as np

N_CORES = 8
B, T, H, NH = 4, 2048, 256, 8
BLOCKS_PER_CORE = 4
EPS = 1e-5
SCALE = 0.0625  # 1/sqrt(H)

_CACHE = {}
LAST_RESULTS = None
TRACE_TMPDIR = None


def _round_f32r(x):
    """Round fp32 to float32r (11 explicit mantissa bits, RNE) like the DVE does."""
    u = np.ascontiguousarray(x, dtype=np.float32).view(np.uint32)
    low = u & np.uint32(0xFFF)
    up = (low > 0x800) | ((low == 0x800) & (((u >> np.uint32(12)) & np.uint32(1)) == 1))
    u = (u & np.uint32(0xFFFFF000)) + np.where(up, np.uint32(0x1000), np.uint32(0)).astype(np.uint32)
    return u.view(np.float32)


def _build():
    import concourse.bacc as bacc
    import concourse.tile as tile
    import concourse.mybir as mybir

    AF = mybir.ActivationFunctionType
    OP = mybir.AluOpType
    AX = mybir.AxisListType
    F32 = mybir.dt.float32
    F32R = mybir.dt.float32r

    nc = bacc.Bacc("TRN2", target_bir_lowering=False, debug=False, num_devices=N_CORES)

    d = {}
    d["xt"] = nc.dram_tensor("xt", [256, 1024], F32R, kind="ExternalInput").ap()
    d["wq"] = nc.dram_tensor("wq", [256, 2048], F32R, kind="ExternalInput").ap()
    d["wk"] = nc.dram_tensor("wk", [256, 2048], F32R, kind="ExternalInput").ap()
    d["wv"] = nc.dram_tensor("wv", [256, 2048], F32R, kind="ExternalInput").ap()
    d["wo"] = nc.dram_tensor("wo", [2048, 256], F32R, kind="ExternalInput").ap()
    d["w1"] = nc.dram_tensor("w1", [256, 1024], F32R, kind="ExternalInput").ap()
    d["w2"] = nc.dram_tensor("w2", [1024, 256], F32R, kind="ExternalInput").ap()
    d["onesm"] = nc.dram_tensor("onesm", [128, 128], F32R, kind="ExternalInput").ap()
    d["ident"] = nc.dram_tensor("ident", [128, 128], F32, kind="ExternalInput").ap()
    d["g1c"] = nc.dram_tensor("g1c", [128, 2], F32, kind="ExternalInput").ap()
    d["b1c"] = nc.dram_tensor("b1c", [128, 2], F32, kind="ExternalInput").ap()
    d["bb1c"] = nc.dram_tensor("bb1c", [128, 8], F32, kind="ExternalInput").ap()
    d["bb2c"] = nc.dram_tensor("bb2c", [128, 2], F32, kind="ExternalInput").ap()
    d["epsc"] = nc.dram_tensor("epsc", [128, 1], F32, kind="ExternalInput").ap()
    d["g2r"] = nc.dram_tensor("g2r", [1, 256], F32R, kind="ExternalInput").ap()
    d["b2r"] = nc.dram_tensor("b2r", [1, 256], F32R, kind="ExternalInput").ap()
    out_d = nc.dram_tensor("out", [1024, 256], F32, kind="ExternalOutput").ap()

    with tile.TileContext(nc) as tc:
        with tc.tile_pool(name="wts", bufs=1) as wp, \
             tc.tile_pool(name="blk", bufs=1) as bp, \
             tc.tile_pool(name="ep", bufs=4) as ep, \
             tc.tile_pool(name="rlp", bufs=3) as rlp, \
             tc.tile_pool(name="rbp", bufs=2) as rbp, \
             tc.tile_pool(name="stp", bufs=1) as stp, \
             tc.tile_pool(name="psA", bufs=2, space="PSUM") as psA, \
             tc.tile_pool(name="psS", bufs=3, space="PSUM") as psS, \
             tc.tile_pool(name="psO", bufs=1, space="PSUM") as psO:

            # ---------------- loads: small consts first so PE starts fast ----
            ones = wp.tile([128, 128], F32R, tag="ones", name="ones")
            nc.sync.dma_start(ones[:], d["onesm"][:])
            ident = wp.tile([128, 128], F32, tag="ident", name="ident")
            nc.sync.dma_start(ident[:], d["ident"][:])
            g1c = wp.tile([128, 2], F32, tag="g1c", name="g1c")
            b1c = wp.tile([128, 2], F32, tag="b1c", name="b1c")
            bb1c = wp.tile([128, 8], F32, tag="bb1c", name="bb1c")
            bb2c = wp.tile([128, 2], F32, tag="bb2c", name="bb2c")
            nc.sync.dma_start(g1c[:], d["g1c"][:])
            nc.sync.dma_start(b1c[:], d["b1c"][:])
            nc.sync.dma_start(bb1c[:], d["bb1c"][:])
            nc.sync.dma_start(bb2c[:], d["bb2c"][:])
            epst = wp.tile([128, 1], F32, tag="epst", name="epst")
            nc.sync.dma_start(epst[:], d["epsc"][:])
            g2r = wp.tile([1, 256], F32R, tag="g2r", name="g2r")
            b2r = wp.tile([1, 256], F32R, tag="b2r", name="b2r")
            nc.sync.dma_start(g2r[:], d["g2r"][:])
            nc.sync.dma_start(b2r[:], d["b2r"][:])

            xt = [wp.tile([128, 1024], F32R, tag=f"xt{i}", name=f"xt{i}") for i in range(2)]
            wq = [wp.tile([128, 2048], F32R, tag=f"wq{i}", name=f"wq{i}") for i in range(2)]
            wk = [wp.tile([128, 2048], F32R, tag=f"wk{i}", name=f"wk{i}") for i in range(2)]
            wv = [wp.tile([128, 2048], F32R, tag=f"wv{i}", name=f"wv{i}") for i in range(2)]
            for i in range(2):
                nc.sync.dma_start(xt[i][:, 0:256], d["xt"][i * 128:(i + 1) * 128, 0:256])
            for c in range(8):
                for i in range(2):
                    nc.sync.dma_start(wq[i][:, c * 256:(c + 1) * 256],
                                      d["wq"][i * 128:(i + 1) * 128, c * 256:(c + 1) * 256])
                    nc.sync.dma_start(wk[i][:, c * 256:(c + 1) * 256],
                                      d["wk"][i * 128:(i + 1) * 128, c * 256:(c + 1) * 256])
            for i in range(2):
                nc.sync.dma_start(xt[i][:, 256:1024], d["xt"][i * 128:(i + 1) * 128, 256:1024])
            for i in range(2):
                nc.sync.dma_start(wv[i][:], d["wv"][i * 128:(i + 1) * 128, :])

            # broadcast g2/b2 across partitions once: ones[0:1,:].T @ row
            g2b = wp.tile([128, 256], F32, tag="g2b", name="g2b")
            b2b = wp.tile([128, 256], F32, tag="b2b", name="b2b")
            for row, dst in ((g2r, g2b), (b2r, b2b)):
                pb = psA.tile([128, 256], F32, tag="mm", name="mm")
                nc.tensor.matmul(pb[:], ones[0:1, :], row[:], start=True, stop=True)
                nc.vector.tensor_copy(dst[:], pb[:])

            # bulkier weights, needed only from the Wo / FFN phase onwards
            wo = wp.tile([128, 4096], F32R, tag="wo", name="wo")
            for kc in range(16):
                nc.sync.dma_start(wo[:, kc * 256:(kc + 1) * 256],
                                  d["wo"][kc * 128:(kc + 1) * 128, :])
            w1 = [wp.tile([128, 1024], F32R, tag=f"w1{i}", name=f"w1{i}") for i in range(2)]
            for i in range(2):
                nc.sync.dma_start(w1[i][:], d["w1"][i * 128:(i + 1) * 128, :])
            w2 = wp.tile([128, 2048], F32R, tag="w2", name="w2")
            for ncc in range(8):
                nc.sync.dma_start(w2[:, ncc * 256:(ncc + 1) * 256],
                                  d["w2"][ncc * 128:(ncc + 1) * 128, :])

            blocks = {}

            def qkv_qtkt(j):
                xcol = j * 256
                # per-chunk tiles -> fine-grained dependencies (consumers only
                # wait for the specific chunk's producer, not the whole tensor)
                qt = [[bp.tile([128, 512], F32R, tag=f"qt{i}_{qc}", name=f"qt{i}_{qc}")
                       for qc in range(4)] for i in range(2)]
                kt = [[bp.tile([128, 256], F32R, tag=f"kt{i}_{c}", name=f"kt{i}_{c}")
                       for c in range(8)] for i in range(2)]
                blocks[j] = {"qt": qt, "kt": kt}
                for c in range(8):
                    for hc in range(2):
                        col = c * 256 + hc * 128
                        pq = psA.tile([128, 256], F32, tag="mm", name="mm")
                        for hic in range(2):
                            nc.tensor.matmul(pq[:], wq[hic][:, col:col + 128],
                                             xt[hic][:, xcol:xcol + 256],
                                             start=(hic == 0), stop=(hic == 1))
                        qcopy = nc.scalar.copy if j == 0 else nc.vector.tensor_copy
                        qcopy(qt[hc][c // 2][:, (c % 2) * 256:(c % 2) * 256 + 256], pq[:])
                        pk = psA.tile([128, 256], F32, tag="mm", name="mm")
                        for hic in range(2):
                            nc.tensor.matmul(pk[:], wk[hic][:, col:col + 128],
                                             xt[hic][:, xcol:xcol + 256],
                                             start=(hic == 0), stop=(hic == 1))
                        nc.vector.tensor_copy(kt[hc][c][:], pk[:])

            def qkv_av(j):
                xcol = j * 256
                av = [[bp.tile([128, 512], F32R, tag=f"av{i}_{nc4}", name=f"av{i}_{nc4}")
                       for nc4 in range(4)] for i in range(2)]
                blocks[j]["av"] = av
                for rt in range(2):
                    for nc4 in range(4):
                        pv = psA.tile([128, 512], F32, tag="mm", name="mm")
                        for hic in range(2):
                            nc.tensor.matmul(pv[:], xt[hic][:, xcol + rt * 128:xcol + rt * 128 + 128],
                                             wv[hic][:, nc4 * 512:(nc4 + 1) * 512],
                                             start=(hic == 0), stop=(hic == 1))
                        nc.scalar.copy(av[rt][nc4][:], pv[:])

            def attention(j):
                qt, kt, av = blocks[j]["qt"], blocks[j]["kt"], blocks[j]["av"]
                onorm = [[bp.tile([128, 512], F32R, tag=f"on{i}_{qc}", name=f"on{i}_{qc}")
                          for qc in range(4)] for i in range(2)]
                blocks[j]["onorm"] = onorm
                for qc in range(4):
                    o0 = psO.tile([128, 512], F32, tag="o0", name="o0")
                    o1 = psO.tile([128, 512], F32, tag="o1", name="o1")
                    dn = psO.tile([128, 512], F32, tag="d", name="d")
                    es = {}

                    def avd(k):
                        c_, rt_ = k // 2, k % 2
                        e = es.pop(k)
                        acol = c_ * 256 + 0 - (c_ // 2) * 512
                        nc.tensor.matmul(dn[:], ones[:], e[:],
                                         start=(k == 0), stop=(k == 15))
                        nc.tensor.matmul(o0[:], av[rt_][c_ // 2][:, acol:acol + 128], e[:],
                                         start=(k == 0), stop=(k == 15))
                        nc.tensor.matmul(o1[:], av[rt_][c_ // 2][:, acol + 128:acol + 256], e[:],
                                         start=(k == 0), stop=(k == 15))

                    # software-pipelined: S/exp run 3 k-tiles ahead of the
                    # dependent AV/denominator matmuls so PE never waits on ACT
                    for kc in range(16):
                        s = psS.tile([128, 512], F32, tag="s", name="s")
                        nc.tensor.matmul(s[:], kt[0][kc // 2][:, (kc % 2) * 128:(kc % 2) * 128 + 128],
                                         qt[0][qc][:],
                                         start=True, stop=False)
                        nc.tensor.matmul(s[:], kt[1][kc // 2][:, (kc % 2) * 128:(kc % 2) * 128 + 128],
                                         qt[1][qc][:],
                                         start=False, stop=True)
                        e = ep.tile([128, 512], F32R, tag="e", name="e")
                        nc.scalar.activation(e[:], s[:], AF.Exp, scale=SCALE)
                        es[kc] = e
                        if kc >= 3:
                            avd(kc - 3)
                    avd(13)
                    avd(14)
                    avd(15)
                    rb = rbp.tile([128, 512], F32, tag="rb", name="rb")
                    nc.vector.reciprocal(rb[:], dn[:])
                    nc.vector.tensor_tensor(onorm[0][qc][:], o0[:], rb[:], op=OP.mult)
                    nc.vector.tensor_tensor(onorm[1][qc][:], o1[:], rb[:], op=OP.mult)

            def wo_z1(j):
                xcol = j * 256
                onorm = blocks[j]["onorm"]
                z1 = [bp.tile([128, 256], F32R, tag=f"z1{i}", name=f"z1{i}") for i in range(2)]
                blocks[j]["z1"] = z1
                for hoc in range(2):
                    ph = psA.tile([128, 256], F32, tag="mm", name="mm")
                    for kc in range(16):
                        c_, h2c = kc // 2, kc % 2
                        ocol = (c_ % 2) * 256
                        nc.tensor.matmul(ph[:], wo[:, kc * 256 + hoc * 128:kc * 256 + hoc * 128 + 128],
                                         onorm[h2c][c_ // 2][:, ocol:ocol + 256],
                                         start=(kc == 0), stop=(kc == 15))
                    nc.vector.tensor_add(z1[hoc][:], ph[:], xt[hoc][:, xcol:xcol + 256])

            def ln1_stats(j):
                z1 = blocks[j]["z1"]
                sq = [bp.tile([128, 256], F32R, tag=f"sq{i}", name=f"sq{i}") for i in range(2)]
                nc.scalar.square(sq[0][:], z1[0][:])
                nc.scalar.square(sq[1][:], z1[1][:])
                ssum = psA.tile([128, 256], F32, tag="mm", name="mm")
                nc.tensor.matmul(ssum[:], ones[:], z1[0][:], start=True, stop=False)
                nc.tensor.matmul(ssum[:], ones[:], z1[1][:], start=False, stop=True)
                ssq = psA.tile([128, 256], F32, tag="mm", name="mm")
                nc.tensor.matmul(ssq[:], ones[:], sq[0][:], start=True, stop=False)
                nc.tensor.matmul(ssq[:], ones[:], sq[1][:], start=False, stop=True)
                blocks[j]["ssum"] = ssum
                blocks[j]["ssq"] = ssq

            def ln1_norm(j):
                z1 = blocks[j]["z1"]
                ssum, ssq = blocks[j]["ssum"], blocks[j]["ssq"]
                mu_b = stp.tile([128, 256], F32, tag="mu", name="mu")
                nc.vector.tensor_scalar_mul(mu_b[:], ssum[:], 1.0 / 256.0)
                mu2 = stp.tile([128, 256], F32, tag="mu2", name="mu2")
                nc.vector.tensor_tensor(mu2[:], mu_b[:], mu_b[:], op=OP.mult)
                var = stp.tile([128, 256], F32, tag="var", name="var")
                nc.vector.scalar_tensor_tensor(var[:], ssq[:], 1.0 / 256.0, mu2[:],
                                               op0=OP.mult, op1=OP.subtract)
                sd = stp.tile([128, 256], F32, tag="sd", name="sd")
                nc.scalar.activation(sd[:], var[:], AF.Sqrt, bias=epst[:])
                rsg = stp.tile([128, 256], F32, tag="rsg", name="rsg")
                nc.vector.reciprocal(rsg[:], sd[:])
                hn = [bp.tile([128, 256], F32R, tag=f"hn{i}", name=f"hn{i}") for i in range(2)]
                blocks[j]["hn"] = hn
                for hc in range(2):
                    t1 = stp.tile([128, 256], F32, tag="t1", name="t1")
                    nc.vector.tensor_tensor(t1[:], z1[hc][:], mu_b[:], op=OP.subtract)
                    t2 = stp.tile([128, 256], F32, tag="t2", name="t2")
                    nc.vector.tensor_tensor(t2[:], t1[:], rsg[:], op=OP.mult)
                    nc.vector.tensor_scalar(hn[hc][:], t2[:],
                                            g1c[:, hc:hc + 1], b1c[:, hc:hc + 1],
                                            op0=OP.mult, op1=OP.add)

            def ffn(j):
                hn = blocks[j]["hn"]
                z2p = [psO.tile([128, 256], F32, tag=t, name=t) for t in ("o0", "o1")]
                for ncc in range(8):
                    pf = psS.tile([128, 256], F32, tag="s", name="s")
                    for hic in range(2):
                        nc.tensor.matmul(pf[:], w1[hic][:, ncc * 128:(ncc + 1) * 128],
                                         hn[hic][:], start=(hic == 0), stop=(hic == 1))
                    rl = rlp.tile([128, 256], F32R, tag="rl", name="rl")
                    nc.scalar.activation(rl[:], pf[:], AF.Relu, bias=bb1c[:, ncc:ncc + 1])
                    nc.tensor.matmul(z2p[0][:], w2[:, ncc * 256:ncc * 256 + 128], rl[:],
                                     start=(ncc == 0), stop=(ncc == 7))
                    nc.tensor.matmul(z2p[1][:], w2[:, ncc * 256 + 128:(ncc + 1) * 256], rl[:],
                                     start=(ncc == 0), stop=(ncc == 7))
                z2 = [bp.tile([128, 256], F32, tag=f"z2{i}", name=f"z2{i}") for i in range(2)]
                blocks[j]["z2"] = z2
                for hoc in range(2):
                    nc.vector.scalar_tensor_tensor(z2[hoc][:], z2p[hoc][:],
                                                   bb2c[:, hoc:hoc + 1], hn[hoc][:],
                                                   op0=OP.add, op1=OP.add)

            def ln2_out(j):
                z2 = blocks[j]["z2"]
                zr, srow, sqs, ssqr, mur, mu2r, varr, sdr, rsr, tt, tg, ot = \
                    ({} for _ in range(12))
                for rt in range(2):
                    zr[rt] = stp.tile([128, 256], F32, tag=f"zr{rt}", name=f"zr{rt}")
                    for hoc in range(2):
                        pt = psS.tile([128, 128], F32, tag="s", name="s")
                        nc.tensor.transpose(pt[:], z2[hoc][:, rt * 128:(rt + 1) * 128], ident[:])
                        nc.vector.tensor_copy(zr[rt][:, hoc * 128:(hoc + 1) * 128], pt[:])
                for rt in range(2):
                    srow[rt] = stp.tile([128, 1], F32, tag=f"srow{rt}", name=f"srow{rt}")
                    nc.vector.reduce_sum(srow[rt][:], zr[rt][:], axis=AX.X)
                    sqs[rt] = stp.tile([128, 256], F32, tag=f"sqs{rt}", name=f"sqs{rt}")
                    ssqr[rt] = stp.tile([128, 1], F32, tag=f"ssqr{rt}", name=f"ssqr{rt}")
                    nc.scalar.activation(sqs[rt][:], zr[rt][:], AF.Square, accum_out=ssqr[rt][:])
                for rt in range(2):
                    mur[rt] = stp.tile([128, 1], F32, tag=f"mur{rt}", name=f"mur{rt}")
                    nc.vector.tensor_scalar_mul(mur[rt][:], srow[rt][:], 1.0 / 256.0)
                    mu2r[rt] = stp.tile([128, 1], F32, tag=f"mu2r{rt}", name=f"mu2r{rt}")
                    nc.vector.tensor_tensor(mu2r[rt][:], mur[rt][:], mur[rt][:], op=OP.mult)
                for rt in range(2):
                    varr[rt] = stp.tile([128, 1], F32, tag=f"varr{rt}", name=f"varr{rt}")
                    nc.vector.scalar_tensor_tensor(varr[rt][:], ssqr[rt][:], 1.0 / 256.0, mu2r[rt][:],
                                                   op0=OP.mult, op1=OP.subtract)
                for rt in range(2):
                    sdr[rt] = stp.tile([128, 1], F32, tag=f"sdr{rt}", name=f"sdr{rt}")
                    nc.scalar.activation(sdr[rt][:], varr[rt][:], AF.Sqrt, bias=epst[:])
                for rt in range(2):
                    rsr[rt] = stp.tile([128, 1], F32, tag=f"rsr{rt}", name=f"rsr{rt}")
                    nc.vector.reciprocal(rsr[rt][:], sdr[rt][:])
                for rt in range(2):
                    tt[rt] = stp.tile([128, 256], F32, tag=f"tt{rt}", name=f"tt{rt}")
                    nc.vector.tensor_scalar(tt[rt][:], zr[rt][:], mur[rt][:], rsr[rt][:],
                                            op0=OP.subtract, op1=OP.mult)
                    tg[rt] = stp.tile([128, 256], F32, tag=f"tg{rt}", name=f"tg{rt}")
                    nc.vector.tensor_tensor(tg[rt][:], tt[rt][:], g2b[:], op=OP.mult)
                    ot[rt] = stp.tile([128, 256], F32, tag=f"ot{rt}", name=f"ot{rt}")
                    nc.vector.tensor_tensor(ot[rt][:], tg[rt][:], b2b[:], op=OP.add)
                    nc.sync.dma_start(out_d[j * 256 + rt * 128:j * 256 + (rt + 1) * 128, :], ot[rt][:])

            # ---------------- schedule ----------------
            # Emission order = per-engine program order.  LN1's serial chain is
            # emitted BEFORE block j+1's QKV copies so it sits early on the DVE
            # queue and finishes while the PE chews on the QKV matmuls; the
            # next attention is emitted before ln2_out so the transposes never
            # head-block the PE.
            qkv_qtkt(0)
            qkv_av(0)
            attention(0)
            for j in range(BLOCKS_PER_CORE):
                if j > 0:
                    ln2_out(j - 1)   # transposes fill the attention->Wo drain
                wo_z1(j)
                ln1_stats(j)
                ln1_norm(j)
                if j + 1 < BLOCKS_PER_CORE:
                    qkv_qtkt(j + 1)
                    qkv_av(j + 1)
                ffn(j)
                if j + 1 < BLOCKS_PER_CORE:
                    attention(j + 1)
            ln2_out(BLOCKS_PER_CORE - 1)

    nc.compile()
    return nc


def kernel(x, Wq, Wk, Wv, Wo, g1, b1, W1, bb1, W2, bb2, g2, b2):
    from concourse import bass_utils
    global LAST_RESULTS

    if "nc" not in _CACHE:
        _CACHE["nc"] = _build()
    nc = _CACHE["nc"]

    x = np.ascontiguousarray(np.asarray(x, dtype=np.float32))
    shared = {
        "wq": _round_f32r(np.asarray(Wq, np.float32)),
        "wk": _round_f32r(np.asarray(Wk, np.float32)),
        "wv": _round_f32r(np.asarray(Wv, np.float32)),
        "wo": _round_f32r(np.asarray(Wo, np.float32)),
        "w1": _round_f32r(np.asarray(W1, np.float32)),
        "w2": _round_f32r(np.asarray(W2, np.float32)),
        "onesm": np.ones((128, 128), np.float32),
        "ident": np.eye(128, dtype=np.float32),
        "g1c": np.ascontiguousarray(np.asarray(g1, np.float32).reshape(2, 128).T),
        "b1c": np.ascontiguousarray(np.asarray(b1, np.float32).reshape(2, 128).T),
        "bb1c": np.ascontiguousarray(np.asarray(bb1, np.float32).reshape(8, 128).T),
        "bb2c": np.ascontiguousarray(np.asarray(bb2, np.float32).reshape(2, 128).T),
        "epsc": np.full((128, 1), EPS, np.float32),
        "g2r": _round_f32r(np.asarray(g2, np.float32).reshape(1, 256)),
        "b2r": _round_f32r(np.asarray(b2, np.float32).reshape(1, 256)),
    }

    in_maps = []
    for c in range(N_CORES):
        xt = np.empty((256, 1024), np.float32)
        for j in range(BLOCKS_PER_CORE):
            g = c * BLOCKS_PER_CORE + j
            b_, n_ = g // NH, g % NH
            xt[:, j * 256:(j + 1) * 256] = x[b_, n_ * 256:(n_ + 1) * 256, :].T
        m = dict(shared)
        m["xt"] = _round_f32r(xt)
        in_maps.append(m)

    kwargs = {}
    if TRACE_TMPDIR is not None:
        kwargs["tmpdir"] = TRACE_TMPDIR
    res = bass_utils.run_bass_kernel_spmd(nc, in_maps, core_ids=list(range(N_CORES)), **kwargs)
    LAST_RESULTS = res

    out = np.empty((B, T, H), np.float32)
    for c in range(N_CORES):
        o = res.results[c]["out"]
        for j in range(BLOCKS_PER_CORE):
            g = c * BLOCKS_PER_CORE + j
            b_, n_ = g // NH, g % NH
            out[b_, n_ * 256:(n_ + 1) * 256, :] = o[j * 256:(j + 1) * 256, :]
    return out


# revision 12
# speedup vs baseline: 1.1474x; 1.1079x over previous
"""Trainium2 Bass kernel for nn_MultiHeadAttention_69088843923801.

Key structural fact: the reference reshapes (B, T, nh*H) -> (B, nh, T, H) as a
raw row-major reinterpretation.  Head n therefore only ever touches x rows
[n*256, (n+1)*256), and the whole layer decomposes into B*nh = 32 fully
independent 256-row blocks (attention is the only cross-row op and it stays
inside a block; LN/FFN are row-wise).  We run 4 blocks per NeuronCore on 8
cores: pure data parallelism, no collectives, no redundant compute.

Per block (x_n = 256 rows of x):
  A_q = x_n @ Wq  -> Qg = A_q.reshape(2048, 256)   (same for K, V)
  S   = Qg @ Kg.T / 16 ; A = softmax(S) ; Og = A @ Vg
  h_attn = Og.reshape(256, 2048) @ Wo
  h = LN(x_n + h_attn); out_rows = LN(h + relu(h@W1 + bb1)@W2 + bb2)

On-chip we enumerate the 2048 "derived tokens" as k' = c*256 + r (c = column
block of the 2048-wide projection, r = row in the block) which makes every
matmul operand a contiguous slice.  Matmuls run in float32r (TF32-like, 11-bit
mantissa, full PE rate at N>=256); accumulation is fp32 in PSUM.

The emission order software-pipelines blocks: block j+1's Q/K projections are
emitted inside block j's LayerNorm1 window so the PE never idles on the LN
serial chain.
"""

import sys

sys.path.insert(0, "/opt/trn_rl_repo")

import numpy as np

N_CORES = 8
B, T, H, NH = 4, 2048, 256, 8
BLOCKS_PER_CORE = 4
EPS = 1e-5
SCALE = 0.0625  # 1/sqrt(H)

_CACHE = {}
LAST_RESULTS = None
TRACE_TMPDIR = None


def _round_f32r(x):
    """Round fp32 to float32r (11 explicit mantissa bits, RNE) like the DVE does."""
    u = np.ascontiguousarray(x, dtype=np.float32).view(np.uint32)
    low = u & np.uint32(0xFFF)
    up = (low > 0x800) | ((low == 0x800) & (((u >> np.uint32(12)) & np.uint32(1)) == 1))
    u = (u & np.uint32(0xFFFFF000)) + np.where(up, np.uint32(0x1000), np.uint32(0)).astype(np.uint32)
    return u.view(np.float32)


def _build():
    import concourse.bacc as bacc
    import concourse.tile as tile
    import concourse.mybir as mybir

    AF = mybir.ActivationFunctionType
    OP = mybir.AluOpType
    AX = mybir.AxisListType
    F32 = mybir.dt.float32
    F32R = mybir.dt.float32r

    nc = bacc.Bacc("TRN2", target_bir_lowering=False, debug=False, num_devices=N_CORES)

    d = {}
    d["xt"] = nc.dram_tensor("xt", [256, 1024], F32R, kind="ExternalInput").ap()
    d["wq"] = nc.dram_tensor("wq", [256, 2048], F32R, kind="ExternalInput").ap()
    d["wk"] = nc.dram_tensor("wk", [256, 2048], F32R, kind="ExternalInput").ap()
    d["wv"] = nc.dram_tensor("wv", [256, 2048], F32R, kind="ExternalInput").ap()
    d["wo"] = nc.dram_tensor("wo", [2048, 256], F32R, kind="ExternalInput").ap()
    d["w1"] = nc.dram_tensor("w1", [256, 1024], F32R, kind="ExternalInput").ap()
    d["w2"] = nc.dram_tensor("w2", [1024, 256], F32R, kind="ExternalInput").ap()
    d["onesm"] = nc.dram_tensor("onesm", [128, 128], F32R, kind="ExternalInput").ap()
    d["ident"] = nc.dram_tensor("ident", [128, 128], F32, kind="ExternalInput").ap()
    d["g1c"] = nc.dram_tensor("g1c", [128, 2], F32, kind="ExternalInput").ap()
    d["b1c"] = nc.dram_tensor("b1c", [128, 2], F32, kind="ExternalInput").ap()
    d["bb1c"] = nc.dram_tensor("bb1c", [128, 8], F32, kind="ExternalInput").ap()
    d["bb2c"] = nc.dram_tensor("bb2c", [128, 2], F32, kind="ExternalInput").ap()
    d["epsc"] = nc.dram_tensor("epsc", [128, 1], F32, kind="ExternalInput").ap()
    d["g2r"] = nc.dram_tensor("g2r", [1, 256], F32R, kind="ExternalInput").ap()
    d["b2r"] = nc.dram_tensor("b2r", [1, 256], F32R, kind="ExternalInput").ap()
    out_d = nc.dram_tensor("out", [1024, 256], F32, kind="ExternalOutput").ap()

    with tile.TileContext(nc) as tc:
        with tc.tile_pool(name="wts", bufs=1) as wp, \
             tc.tile_pool(name="blk", bufs=1) as bp, \
             tc.tile_pool(name="ep", bufs=4) as ep, \
             tc.tile_pool(name="rlp", bufs=3) as rlp, \
             tc.tile_pool(name="rbp", bufs=2) as rbp, \
             tc.tile_pool(name="stp", bufs=1) as stp, \
             tc.tile_pool(name="psA", bufs=2, space="PSUM") as psA, \
             tc.tile_pool(name="psS", bufs=3, space="PSUM") as psS, \
             tc.tile_pool(name="psO", bufs=1, space="PSUM") as psO:

            # ---------------- loads: small consts first so PE starts fast ----
            ones = wp.tile([128, 128], F32R, tag="ones", name="ones")
            nc.sync.dma_start(ones[:], d["onesm"][:])
            ident = wp.tile([128, 128], F32, tag="ident", name="ident")
            nc.sync.dma_start(ident[:], d["ident"][:])
            g1c = wp.tile([128, 2], F32, tag="g1c", name="g1c")
            b1c = wp.tile([128, 2], F32, tag="b1c", name="b1c")
            bb1c = wp.tile([128, 8], F32, tag="bb1c", name="bb1c")
            bb2c = wp.tile([128, 2], F32, tag="bb2c", name="bb2c")
            nc.sync.dma_start(g1c[:], d["g1c"][:])
            nc.sync.dma_start(b1c[:], d["b1c"][:])
            nc.sync.dma_start(bb1c[:], d["bb1c"][:])
            nc.sync.dma_start(bb2c[:], d["bb2c"][:])
            epst = wp.tile([128, 1], F32, tag="epst", name="epst")
            nc.sync.dma_start(epst[:], d["epsc"][:])
            g2r = wp.tile([1, 256], F32R, tag="g2r", name="g2r")
            b2r = wp.tile([1, 256], F32R, tag="b2r", name="b2r")
            nc.sync.dma_start(g2r[:], d["g2r"][:])
            nc.sync.dma_start(b2r[:], d["b2r"][:])

            xt = [wp.tile([128, 1024], F32R, tag=f"xt{i}", name=f"xt{i}") for i in range(2)]
            wq = [wp.tile([128, 2048], F32R, tag=f"wq{i}", name=f"wq{i}") for i in range(2)]
            wk = [wp.tile([128, 2048], F32R, tag=f"wk{i}", name=f"wk{i}") for i in range(2)]
            wv = [wp.tile([128, 2048], F32R, tag=f"wv{i}", name=f"wv{i}") for i in range(2)]
            for i in range(2):
                nc.sync.dma_start(xt[i][:, 0:256], d["xt"][i * 128:(i + 1) * 128, 0:256])
            for c in range(8):
                for i in range(2):
                    nc.sync.dma_start(wq[i][:, c * 256:(c + 1) * 256],
                                      d["wq"][i * 128:(i + 1) * 128, c * 256:(c + 1) * 256])
                    nc.sync.dma_start(wk[i][:, c * 256:(c + 1) * 256],
                                      d["wk"][i * 128:(i + 1) * 128, c * 256:(c + 1) * 256])
            for i in range(2):
                nc.sync.dma_start(xt[i][:, 256:1024], d["xt"][i * 128:(i + 1) * 128, 256:1024])
            for i in range(2):
                nc.sync.dma_start(wv[i][:], d["wv"][i * 128:(i + 1) * 128, :])

            # broadcast g2/b2 across partitions once: ones[0:1,:].T @ row
            g2b = wp.tile([128, 256], F32, tag="g2b", name="g2b")
            b2b = wp.tile([128, 256], F32, tag="b2b", name="b2b")
            for row, dst in ((g2r, g2b), (b2r, b2b)):
                pb = psA.tile([128, 256], F32, tag="mm", name="mm")
                nc.tensor.matmul(pb[:], ones[0:1, :], row[:], start=True, stop=True)
                nc.vector.tensor_copy(dst[:], pb[:])

            # bulkier weights, needed only from the Wo / FFN phase onwards
            wo = wp.tile([128, 4096], F32R, tag="wo", name="wo")
            for kc in range(16):
                nc.sync.dma_start(wo[:, kc * 256:(kc + 1) * 256],
                                  d["wo"][kc * 128:(kc + 1) * 128, :])
            w1 = [wp.tile([128, 1024], F32R, tag=f"w1{i}", name=f"w1{i}") for i in range(2)]
            for i in range(2):
                nc.sync.dma_start(w1[i][:], d["w1"][i * 128:(i + 1) * 128, :])
            w2 = wp.tile([128, 2048], F32R, tag="w2", name="w2")
            for ncc in range(8):
                nc.sync.dma_start(w2[:, ncc * 256:(ncc + 1) * 256],
                                  d["w2"][ncc * 128:(ncc + 1) * 128, :])

            blocks = {}

            def qkv_qtkt(j):
                xcol = j * 256
                # per-chunk tiles -> fine-grained dependencies (consumers only
                # wait for the specific chunk's producer, not the whole tensor)
                qt = [[bp.tile([128, 512], F32R, tag=f"qt{i}_{qc}", name=f"qt{i}_{qc}")
                       for qc in range(4)] for i in range(2)]
                kt = [[bp.tile([128, 256], F32R, tag=f"kt{i}_{c}", name=f"kt{i}_{c}")
                       for c in range(8)] for i in range(2)]
                blocks[j] = {"qt": qt, "kt": kt}
                for c in range(8):
                    for hc in range(2):
                        col = c * 256 + hc * 128
                        pq = psA.tile([128, 256], F32, tag="mm", name="mm")
                        for hic in range(2):
                            nc.tensor.matmul(pq[:], wq[hic][:, col:col + 128],
                                             xt[hic][:, xcol:xcol + 256],
                                             start=(hic == 0), stop=(hic == 1))
                        qcopy = nc.scalar.copy if j == 0 else nc.vector.tensor_copy
                        qcopy(qt[hc][c // 2][:, (c % 2) * 256:(c % 2) * 256 + 256], pq[:])
                        pk = psA.tile([128, 256], F32, tag="mm", name="mm")
                        for hic in range(2):
                            nc.tensor.matmul(pk[:], wk[hic][:, col:col + 128],
                                             xt[hic][:, xcol:xcol + 256],
                                             start=(hic == 0), stop=(hic == 1))
                        nc.vector.tensor_copy(kt[hc][c][:], pk[:])

            def qkv_av(j):
                xcol = j * 256
                av = [[bp.tile([128, 512], F32R, tag=f"av{i}_{nc4}", name=f"av{i}_{nc4}")
                       for nc4 in range(4)] for i in range(2)]
                blocks[j]["av"] = av
                for rt in range(2):
                    for nc4 in range(4):
                        pv = psA.tile([128, 512], F32, tag="mm", name="mm")
                        for hic in range(2):
                            nc.tensor.matmul(pv[:], xt[hic][:, xcol + rt * 128:xcol + rt * 128 + 128],
                                             wv[hic][:, nc4 * 512:(nc4 + 1) * 512],
                                             start=(hic == 0), stop=(hic == 1))
                        nc.scalar.copy(av[rt][nc4][:], pv[:])

            def attention(j):
                qt, kt, av = blocks[j]["qt"], blocks[j]["kt"], blocks[j]["av"]
                onorm = [[bp.tile([128, 512], F32R, tag=f"on{i}_{qc}", name=f"on{i}_{qc}")
                          for qc in range(4)] for i in range(2)]
                blocks[j]["onorm"] = onorm
                for qc in range(4):
                    o0 = psO.tile([128, 512], F32, tag="o0", name="o0")
                    o1 = psO.tile([128, 512], F32, tag="o1", name="o1")
                    dn = psO.tile([128, 512], F32, tag="d", name="d")
                    es = {}

                    def avd(k):
                        c_, rt_ = k // 2, k % 2
                        e = es.pop(k)
                        acol = c_ * 256 + 0 - (c_ // 2) * 512
                        nc.tensor.matmul(dn[:], ones[:], e[:],
                                         start=(k == 0), stop=(k == 15))
                        nc.tensor.matmul(o0[:], av[rt_][c_ // 2][:, acol:acol + 128], e[:],
                                         start=(k == 0), stop=(k == 15))
                        nc.tensor.matmul(o1[:], av[rt_][c_ // 2][:, acol + 128:acol + 256], e[:],
                                         start=(k == 0), stop=(k == 15))

                    # software-pipelined: S/exp run 3 k-tiles ahead of the
                    # dependent AV/denominator matmuls so PE never waits on ACT
                    for kc in range(16):
                        s = psS.tile([128, 512], F32, tag="s", name="s")
                        nc.tensor.matmul(s[:], kt[0][kc // 2][:, (kc % 2) * 128:(kc % 2) * 128 + 128],
                                         qt[0][qc][:],
                                         start=True, stop=False)
                        nc.tensor.matmul(s[:], kt[1][kc // 2][:, (kc % 2) * 128:(kc % 2) * 128 + 128],
                                         qt[1][qc][:],
                                         start=False, stop=True)
                        e = ep.tile([128, 512], F32R, tag="e", name="e")
                        nc.scalar.activation(e[:], s[:], AF.Exp, scale=SCALE)
                        es[kc] = e
                        if kc >= 3:
                            avd(kc - 3)
                    avd(13)
                    avd(14)
                    avd(15)
                    rb = rbp.tile([128, 512], F32, tag="rb", name="rb")
                    nc.vector.reciprocal_approx_fast(rb[:], dn[:])
                    nc.vector.tensor_tensor(onorm[0][qc][:], o0[:], rb[:], op=OP.mult)
                    nc.vector.tensor_tensor(onorm[1][qc][:], o1[:], rb[:], op=OP.mult)

            def wo_z1(j):
                xcol = j * 256
                onorm = blocks[j]["onorm"]
                z1 = [bp.tile([128, 256], F32R, tag=f"z1{i}", name=f"z1{i}") for i in range(2)]
                blocks[j]["z1"] = z1
                for hoc in range(2):
                    ph = psA.tile([128, 256], F32, tag="mm", name="mm")
                    for kc in range(16):
                        c_, h2c = kc // 2, kc % 2
                        ocol = (c_ % 2) * 256
                        nc.tensor.matmul(ph[:], wo[:, kc * 256 + hoc * 128:kc * 256 + hoc * 128 + 128],
                                         onorm[h2c][c_ // 2][:, ocol:ocol + 256],
                                         start=(kc == 0), stop=(kc == 15))
                    nc.vector.tensor_add(z1[hoc][:], ph[:], xt[hoc][:, xcol:xcol + 256])

            def ln1_stats(j):
                z1 = blocks[j]["z1"]
                sq = [bp.tile([128, 256], F32R, tag=f"sq{i}", name=f"sq{i}") for i in range(2)]
                nc.scalar.square(sq[0][:], z1[0][:])
                nc.scalar.square(sq[1][:], z1[1][:])
                ssum = psA.tile([128, 256], F32, tag="mm", name="mm")
                nc.tensor.matmul(ssum[:], ones[:], z1[0][:], start=True, stop=False)
                nc.tensor.matmul(ssum[:], ones[:], z1[1][:], start=False, stop=True)
                ssq = psA.tile([128, 256], F32, tag="mm", name="mm")
                nc.tensor.matmul(ssq[:], ones[:], sq[0][:], start=True, stop=False)
                nc.tensor.matmul(ssq[:], ones[:], sq[1][:], start=False, stop=True)
                blocks[j]["ssum"] = ssum
                blocks[j]["ssq"] = ssq

            def ln1_norm(j):
                z1 = blocks[j]["z1"]
                ssum, ssq = blocks[j]["ssum"], blocks[j]["ssq"]
                mu_b = stp.tile([128, 256], F32, tag="mu", name="mu")
                nc.vector.tensor_scalar_mul(mu_b[:], ssum[:], 1.0 / 256.0)
                mu2 = stp.tile([128, 256], F32, tag="mu2", name="mu2")
                nc.vector.tensor_tensor(mu2[:], mu_b[:], mu_b[:], op=OP.mult)
                var = stp.tile([128, 256], F32, tag="var", name="var")
                nc.vector.scalar_tensor_tensor(var[:], ssq[:], 1.0 / 256.0, mu2[:],
                                               op0=OP.mult, op1=OP.subtract)
                sd = stp.tile([128, 256], F32, tag="sd", name="sd")
                nc.scalar.activation(sd[:], var[:], AF.Sqrt, bias=epst[:])
                rsg = stp.tile([128, 256], F32, tag="rsg", name="rsg")
                nc.vector.reciprocal_approx_fast(rsg[:], sd[:])
                hn = [bp.tile([128, 256], F32R, tag=f"hn{i}", name=f"hn{i}") for i in range(2)]
                blocks[j]["hn"] = hn
                for hc in range(2):
                    t1 = stp.tile([128, 256], F32, tag="t1", name="t1")
                    nc.vector.tensor_tensor(t1[:], z1[hc][:], mu_b[:], op=OP.subtract)
                    t2 = stp.tile([128, 256], F32, tag="t2", name="t2")
                    nc.vector.tensor_tensor(t2[:], t1[:], rsg[:], op=OP.mult)
                    nc.vector.tensor_scalar(hn[hc][:], t2[:],
                                            g1c[:, hc:hc + 1], b1c[:, hc:hc + 1],
                                            op0=OP.mult, op1=OP.add)

            def ffn(j):
                hn = blocks[j]["hn"]
                z2p = [psO.tile([128, 256], F32, tag=t, name=t) for t in ("o0", "o1")]
                for ncc in range(8):
                    pf = psS.tile([128, 256], F32, tag="s", name="s")
                    for hic in range(2):
                        nc.tensor.matmul(pf[:], w1[hic][:, ncc * 128:(ncc + 1) * 128],
                                         hn[hic][:], start=(hic == 0), stop=(hic == 1))
                    rl = rlp.tile([128, 256], F32R, tag="rl", name="rl")
                    nc.scalar.activation(rl[:], pf[:], AF.Relu, bias=bb1c[:, ncc:ncc + 1])
                    nc.tensor.matmul(z2p[0][:], w2[:, ncc * 256:ncc * 256 + 128], rl[:],
                                     start=(ncc == 0), stop=(ncc == 7))
                    nc.tensor.matmul(z2p[1][:], w2[:, ncc * 256 + 128:(ncc + 1) * 256], rl[:],
                                     start=(ncc == 0), stop=(ncc == 7))
                z2 = [bp.tile([128, 256], F32, tag=f"z2{i}", name=f"z2{i}") for i in range(2)]
                blocks[j]["z2"] = z2
                for hoc in range(2):
                    nc.vector.scalar_tensor_tensor(z2[hoc][:], z2p[hoc][:],
                                                   bb2c[:, hoc:hoc + 1], hn[hoc][:],
                                                   op0=OP.add, op1=OP.add)

            def ln2_out(j):
                z2 = blocks[j]["z2"]
                zr, srow, sqs, ssqr, mur, mu2r, varr, sdr, rsr, tt, tg, ot = \
                    ({} for _ in range(12))
                for rt in range(2):
                    zr[rt] = stp.tile([128, 256], F32, tag=f"zr{rt}", name=f"zr{rt}")
                    for hoc in range(2):
                        pt = psS.tile([128, 128], F32, tag="s", name="s")
                        nc.tensor.transpose(pt[:], z2[hoc][:, rt * 128:(rt + 1) * 128], ident[:])
                        nc.vector.tensor_copy(zr[rt][:, hoc * 128:(hoc + 1) * 128], pt[:])
                for rt in range(2):
                    srow[rt] = stp.tile([128, 1], F32, tag=f"srow{rt}", name=f"srow{rt}")
                    nc.vector.reduce_sum(srow[rt][:], zr[rt][:], axis=AX.X)
                    sqs[rt] = stp.tile([128, 256], F32, tag=f"sqs{rt}", name=f"sqs{rt}")
                    ssqr[rt] = stp.tile([128, 1], F32, tag=f"ssqr{rt}", name=f"ssqr{rt}")
                    nc.scalar.activation(sqs[rt][:], zr[rt][:], AF.Square, accum_out=ssqr[rt][:])
                for rt in range(2):
                    mur[rt] = stp.tile([128, 1], F32, tag=f"mur{rt}", name=f"mur{rt}")
                    nc.vector.tensor_scalar_mul(mur[rt][:], srow[rt][:], 1.0 / 256.0)
                    mu2r[rt] = stp.tile([128, 1], F32, tag=f"mu2r{rt}", name=f"mu2r{rt}")
                    nc.vector.tensor_tensor(mu2r[rt][:], mur[rt][:], mur[rt][:], op=OP.mult)
                for rt in range(2):
                    varr[rt] = stp.tile([128, 1], F32, tag=f"varr{rt}", name=f"varr{rt}")
                    nc.vector.scalar_tensor_tensor(varr[rt][:], ssqr[rt][:], 1.0 / 256.0, mu2r[rt][:],
                                                   op0=OP.mult, op1=OP.subtract)
                for rt in range(2):
                    sdr[rt] = stp.tile([128, 1], F32, tag=f"sdr{rt}", name=f"sdr{rt}")
                    nc.scalar.activation(sdr[rt][:], varr[rt][:], AF.Sqrt, bias=epst[:])
                for rt in range(2):
                    rsr[rt] = stp.tile([128, 1], F32, tag=f"rsr{rt}", name=f"rsr{rt}")
                    nc.vector.reciprocal_approx_fast(rsr[rt][:], sdr[rt][:])
                for rt in range(2):
                    tt[rt] = stp.tile([128, 256], F32, tag=f"tt{rt}", name=f"tt{rt}")
                    nc.vector.tensor_scalar(tt[rt][:], zr[rt][:], mur[rt][:], rsr[rt][:],
                                            op0=OP.subtract, op1=OP.mult)
                    tg[rt] = stp.tile([128, 256], F32, tag=f"tg{rt}", name=f"tg{rt}")
                    nc.vector.tensor_tensor(tg[rt][:], tt[rt][:], g2b[:], op=OP.mult)
                    ot[rt] = stp.tile([128, 256], F32, tag=f"ot{rt}", name=f"ot{rt}")
                    nc.vector.tensor_tensor(ot[rt][:], tg[rt][:], b2b[:], op=OP.add)
                    nc.sync.dma_start(out_d[j * 256 + rt * 128:j * 256 + (rt + 1) * 128, :], ot[rt][:])

            # ---------------- schedule ----------------
            # Emission order = per-engine program order.  LN1's serial chain is
            # emitted BEFORE block j+1's QKV copies so it sits early on the DVE
            # queue and finishes while the PE chews on the QKV matmuls; the
            # next attention is emitted before ln2_out so the transposes never
            # head-block the PE.
            qkv_qtkt(0)
            qkv_av(0)
            attention(0)
            for j in range(BLOCKS_PER_CORE):
                if j > 0:
                    ln2_out(j - 1)   # transposes fill the attention->Wo drain
                wo_z1(j)
                ln1_stats(j)
                ln1_norm(j)
                if j + 1 < BLOCKS_PER_CORE:
                    qkv_qtkt(j + 1)
                    qkv_av(j + 1)
                ffn(j)
                if j + 1 < BLOCKS_PER_CORE:
                    attention(j + 1)
            ln2_out(BLOCKS_PER_CORE - 1)

    nc.compile()
    return nc


def kernel(x, Wq, Wk, Wv, Wo, g1, b1, W1, bb1, W2, bb2, g2, b2):
    from concourse import bass_utils
    global LAST_RESULTS

    if "nc" not in _CACHE:
        _CACHE["nc"] = _build()
    nc = _CACHE["nc"]

    x = np.ascontiguousarray(np.asarray(x, dtype=np.float32))
    shared = {
        "wq": _round_f32r(np.asarray(Wq, np.float32)),
        "wk": _round_f32r(np.asarray(Wk, np.float32)),
        "wv": _round_f32r(np.asarray(Wv, np.float32)),
        "wo": _round_f32r(np.asarray(Wo, np.float32)),
        "w1": _round_f32r(np.asarray(W1, np.float32)),
        "w2": _round_f32r(np.asarray(W2, np.float32)),
        "onesm": np.ones((128, 128), np.float32),
        "ident": np.eye(128, dtype=np.float32),
        "g1c": np.ascontiguousarray(np.asarray(g1, np.float32).reshape(2, 128).T),
        "b1c": np.ascontiguousarray(np.asarray(b1, np.float32).reshape(2, 128).T),
        "bb1c": np.ascontiguousarray(np.asarray(bb1, np.float32).reshape(8, 128).T),
        "bb2c": np.ascontiguousarray(np.asarray(bb2, np.float32).reshape(2, 128).T),
        "epsc": np.full((128, 1), EPS, np.float32),
        "g2r": _round_f32r(np.asarray(g2, np.float32).reshape(1, 256)),
        "b2r": _round_f32r(np.asarray(b2, np.float32).reshape(1, 256)),
    }

    in_maps = []
    for c in range(N_CORES):
        xt = np.empty((256, 1024), np.float32)
        for j in range(BLOCKS_PER_CORE):
            g = c * BLOCKS_PER_CORE + j
            b_, n_ = g // NH, g % NH
            xt[:, j * 256:(j + 1) * 256] = x[b_, n_ * 256:(n_ + 1) * 256, :].T
        m = dict(shared)
        m["xt"] = _round_f32r(xt)
        in_maps.append(m)

    kwargs = {}
    if TRACE_TMPDIR is not None:
        kwargs["tmpdir"] = TRACE_TMPDIR
    res = bass_utils.run_bass_kernel_spmd(nc, in_maps, core_ids=list(range(N_CORES)), **kwargs)
    LAST_RESULTS = res

    out = np.empty((B, T, H), np.float32)
    for c in range(N_CORES):
        o = res.results[c]["out"]
        for j in range(BLOCKS_PER_CORE):
            g = c * BLOCKS_PER_CORE + j
            b_, n_ = g // NH, g % NH
            out[b_, n_ * 256:(n_ + 1) * 256, :] = o[j * 256:(j + 1) * 256, :]
    return out
